# revision 15
# baseline (speedup 1.0000x reference)
"""Distributed GQA attention kernel for 8 TRN2 NeuronCores — v2.

Problem: B=2, S=2048, DIM=2048, NH=32 q heads, NKV=8 kv heads, HD=64,
RoPE (base 10000), causal mask, out-projection.

Sharding (8 cores): core c -> batch b = c//4, rank r = c%4.
Each core: 8 q heads (as 4 pair-tiles), 2 kv heads, full S. Host
reassembles out[b, r*512:(r+1)*512, :] from core 4*b + r.

v2 structure (vs v1):
  - Host supplies x TRANSPOSED (xT [DIM, S] bf16): no on-chip transposes.
  - v projected directly in [key, dim] orientation (no vT transpose).
  - Scores matmuls run in 128x128 array mode with zero-padded kT copies
    (kTA/kTB) -> no PE tiling-mode switches, uniform warm matmul stream.
  - qi-outer attention interleaved with the NEXT s-chunk's projections:
    proj matmuls feed the PE queue between scores and PV so the PE never
    waits on the exp chain and HAM stays at full clock.
  - Softmax denominators: ones-column in v1 (PV computes sums), then
    reciprocal_approx_fast (DVE) + K=1 broadcast matmul (no DRAM hop).
  - Causal mask multiplies on DVE (gpsimd is reserved for collectives so
    they can't block compute).
  - Per-pair AllGather of normalized A^T overlapped with later pairs.
"""

import os
import numpy as np

import concourse.bass as bass
import concourse.mybir as mybir
from concourse import bacc, tile
from concourse.bass import ds

F32 = mybir.dt.float32
BF16 = mybir.dt.bfloat16
AF = mybir.ActivationFunctionType

# -------- problem constants (full size) --------
B, S, DIM = 2, 2048, 2048
NH, NKV, HD = 32, 8, 64
ROPE_BASE = 10000.0
N_CORES = 8
GROUP = 4                      # cores per batch group
NHL = NH // GROUP              # 8 local q heads
NKVL = NKV // GROUP            # 2 local kv heads
NQT = NHL // 2                 # 4 head-pair tiles
P = 128                        # partitions
NF = 512                       # free-dim tile (one PSUM bank of f32)
N_CHUNKS = S // NF             # 4 s-chunks
D_TILES = DIM // P             # 16 contraction tiles
KT_TOTAL = S // P              # 16 key tiles
OUT_S = S // GROUP             # 512 output rows per core
WO_R = GROUP * NHL * HD        # 2048
WO_C = DIM

LAST_RESULTS = None


def _rope_tables(S_):
    """cos table tiled to 128 partitions, and a sign-folded sin table:
    rows p with p%64 < 32 carry -sin (x1 half), else +sin (x2 half)."""
    inv_freq = 1.0 / (ROPE_BASE ** (np.arange(0, HD, 2, dtype=np.float64) / HD))
    t = np.arange(S_, dtype=np.float64)
    freqs = inv_freq[:, None] * t[None, :]          # [32, S]
    cos32 = np.cos(freqs).astype(np.float32)
    sin32 = np.sin(freqs).astype(np.float32)
    c128 = np.tile(cos32, (4, 1))                   # [128, S]
    s128 = np.tile(np.concatenate([-sin32, sin32], axis=0), (2, 1))
    return c128, s128


def build_nc_v2():
    import ml_dtypes
    nc = bacc.Bacc(None, target_bir_lowering=False, num_devices=N_CORES)

    # All inputs host-retiled so each SBUF destination loads with ONE big
    # dma_start (fixed cost ~1-2us per DMA makes many small loads brutal):
    #   xt[n][p, k*NF+c]   = x[n*NF+c, k*P+p]      (chunk n, d-tile k)
    #   wq[p, k*DQ+c]      = Wq[k*P+p, c]          etc for wk/wv
    #   wo[h][p, i*NF+c]   = Wo[kidx*P+p, n_o*NF+c], i = (n_o-2h)*16+kidx
    xT_p = nc.declare_dram_parameter("xT", [P, N_CHUNKS * D_TILES * NF], BF16,
                                     isOutput=False)
    wq_p = nc.declare_dram_parameter("wq", [P, D_TILES * NHL * HD], BF16,
                                     isOutput=False)
    wk_p = nc.declare_dram_parameter("wk", [P, D_TILES * NKVL * HD], BF16,
                                     isOutput=False)
    wv_p = nc.declare_dram_parameter("wv", [P, D_TILES * NKVL * HD], BF16,
                                     isOutput=False)
    wo_p = nc.declare_dram_parameter("wo", [P, (WO_R // P) * WO_C], BF16,
                                     isOutput=False)
    out_p = nc.declare_dram_parameter("out", [OUT_S, WO_C], F32, isOutput=True)

    cos_np, sin_np = _rope_tables(S)
    cos_d = nc.inline_tensor(cos_np, name="cos_tab")
    sin_d = nc.inline_tensor(sin_np, name="sin_tab")
    ones_bc_np = np.zeros((P, P), dtype=ml_dtypes.bfloat16)
    ones_bc_np[HD, :] = 1.0
    ones_bc_d = nc.inline_tensor(ones_bc_np, name="ones_bc")
    onesv_d = nc.inline_tensor(np.ones((P, 1), dtype=ml_dtypes.bfloat16),
                               name="ones_col")
    xx = np.arange(P)[:, None]
    yy = np.arange(NF)[None, :]
    mask_np = np.stack([(yy - xx - j * P >= 0) for j in range(NF // P)])
    mask2_np = np.stack([np.concatenate([mask_np[j], mask_np[j + 1]], axis=1)
                         for j in (0, 2)])
    mask_d = nc.inline_tensor(mask2_np.astype(ml_dtypes.bfloat16),
                              name="cmask")

    groups = [list(range(g * GROUP, (g + 1) * GROUP))
              for g in range(N_CORES // GROUP)]

    with tile.TileContext(nc) as tc:
        with tc.tile_pool(name="persist", bufs=1) as pp:
            # comm bounce buffers (DRAM)
            dram_pool = tc.tile_pool(name="dram", bufs=1, space="DRAM")
            dp = dram_pool.__enter__()
            # half-gather layout: ag_in[t] rows h*128.. = qi-half h's A^T
            # (cols (qi%2)*512..); ag_out[t] rows h*512.. = gathered half h
            ag_in = [dp.tile([2 * P, 2 * NF], BF16, name=f"ag_in{t}")
                     for t in range(NQT)]
            ag_out = [dp.tile([2 * GROUP * P, 2 * NF], BF16,
                              name=f"ag_out{t}") for t in range(NQT)]
            dum_in = dp.tile([1, 4], F32, name="dum_in")
            dum_out = dp.tile([GROUP, 4], F32, name="dum_out")

            # dummy gather first: absorbs inter-core launch skew while
            # nothing is pending (collectives freeze the model DMA rings,
            # so real gathers should never be the first sync point)
            nc.gpsimd.collective_compute(
                "AllGather", mybir.AluOpType.bypass, replica_groups=groups,
                ins=[dum_in.opt()], outs=[dum_out.opt()])

            ones_bc = pp.tile([P, P], BF16, name="ones_bc_sb")
            cmask = [pp.tile([P, 2 * NF], BF16, name=f"cmask{j}")
                     for j in range(2)]

            def load_consts():
                nc.sync.dma_start(out=ones_bc, in_=ones_bc_d[:, :])
                for j in range(2):
                    nc.sync.dma_start(out=cmask[j], in_=mask_d[j, :, :])

            qT = [pp.tile([P, S], BF16, name=f"qT{t}") for t in range(NQT)]
            # zero-padded per-head k tiles: kTp[0] has head-A dims on
            # partitions 0-63 (zeros elsewhere), kTp[1] head-B on 64-127
            kTp = [pp.tile([P, S], BF16, name=f"kTp{i}") for i in range(2)]
            nc.vector.memset(kTp[0][HD:P, :], 0.0)
            nc.vector.memset(kTp[1][0:HD, :], 0.0)
            # v1 tiles: [128 keys, 130]: cols 0-63 kv0, 64 ones, 65-128 kv1,
            # 129 ones (ones via memset: no tiny DMAs)
            v1 = [pp.tile([P, 2 * (HD + 1)], BF16, name=f"v1_{c}")
                  for c in range(KT_TOTAL)]
            for c in range(KT_TOTAL):
                nc.vector.memset(v1[c][:, HD:HD + 1], 1.0)
                nc.vector.memset(v1[c][:, 2 * HD + 1:2 * HD + 2], 1.0)

            p1_cm = tc.tile_pool(name="p1", bufs=1)
            p1 = p1_cm.__enter__()
            p2_cm = tc.tile_pool(name="p2", bufs=1)
            p2 = p2_cm.__enter__()
            pacc_cm = tc.tile_pool(name="accpsum", bufs=2, space="PSUM")
            pacc = pacc_cm.__enter__()
            psc_cm = tc.tile_pool(name="scpsum", bufs=2, space="PSUM")
            psc = psc_cm.__enter__()
            pot_cm = tc.tile_pool(name="otpsum", bufs=2, space="PSUM")
            pot = pot_cm.__enter__()

            # ---- input loads: one big dma_start per logical input ----
            # xt chunk tile: [128, 16*512]; slice k = d-tile k's s-chunk
            def load_x_chunk(n):
                xs = p2.tile([P, D_TILES * NF], BF16, name="xch", tag="xch",
                             bufs=2)
                nc.sync.dma_start(
                    out=xs, in_=xT_p[:, n * D_TILES * NF:(n + 1) * D_TILES * NF])
                return xs

            DQ = NHL * HD
            DKV = NKVL * HD
            wq_all = p2.tile([P, D_TILES * DQ], BF16, name="wq_all")
            wk_all = p2.tile([P, D_TILES * DKV], BF16, name="wk_all")
            wv_all = p2.tile([P, D_TILES * DKV], BF16, name="wv_all")
            xsl0 = p2.tile([P, D_TILES * NF], BF16, name="xch", tag="xch",
                           bufs=2)
            # x on the sync ring, weights on the scalar ring: the two HWDGE
            # rings run concurrently, halving time-to-first-matmul
            nc.sync.dma_start(out=xsl0, in_=xT_p[:, 0:D_TILES * NF])
            nc.scalar.dma_start(out=wq_all, in_=wq_p[:, :])
            nc.scalar.dma_start(out=wk_all, in_=wk_p[:, :])
            nc.scalar.dma_start(out=wv_all, in_=wv_p[:, :])
            load_consts()
            wq_sb = [wq_all[:, k * DQ:(k + 1) * DQ] for k in range(D_TILES)]
            wk_sb = [wk_all[:, k * DKV:(k + 1) * DKV] for k in range(D_TILES)]
            wv_sb = [wv_all[:, k * DKV:(k + 1) * DKV] for k in range(D_TILES)]
            xsl = {0: xsl0}

            c_all = p2.tile([P, S], F32, name="c_all")
            s_all = p2.tile([P, S], F32, name="s_all")
            nc.scalar.dma_start(out=c_all, in_=cos_d[:, :])
            nc.scalar.dma_start(out=s_all, in_=sin_d[:, :])

            def load_cs(n):
                return (c_all[:, n * NF:(n + 1) * NF],
                        s_all[:, n * NF:(n + 1) * NF])

            # Wo: four resident 2MB quarters (quarter = one n_o column
            # group). Quarters 0/1 load into p1 during attention; 2/3 load
            # into the out-projection pool after the proj-phase pool frees.
            wo_q = {}
            wo_q[0] = p1.tile([P, (WO_R // P) * NF], BF16, name="wo_q0")
            wo_q[1] = p1.tile([P, (WO_R // P) * NF], BF16, name="wo_q1")

            def wo_sb(kidx, n_o):
                return wo_q[n_o][:, kidx * NF:(kidx + 1) * NF]

            def drip_wo(i):
                nc.sync.dma_start(
                    out=wo_q[i],
                    in_=wo_p[:, i * (WO_R // P) * NF:
                             (i + 1) * (WO_R // P) * NF])

            # ---- projection chunk as a generator: yields after each PE op
            def proj_chunk(n):
                ncol = slice(n * NF, (n + 1) * NF)
                xs = xsl[n]
                if n + 1 < N_CHUNKS:
                    xsl[n + 1] = load_x_chunk(n + 1)  # prefetch
                c128, s128 = load_cs(n)
                # k FIRST (its rope gates the next qi block's scores), then
                # v, then q0..q3 (q[t] gates only attention tile t)
                def emit_qk(m):
                    acc = pacc.tile([P, NF], F32, name="acc", tag="acc")
                    for k in range(D_TILES):
                        if m < NQT:
                            lhsT = wq_sb[k][:, m * P:(m + 1) * P]
                        else:
                            lhsT = wk_sb[k]
                        nc.tensor.matmul(acc, lhsT,
                                         xs[:, k * NF:(k + 1) * NF],
                                         start=(k == 0),
                                         stop=(k == D_TILES - 1))
                        yield
                    # rope: out = raw*cos + swap(raw)*sgn_sin
                    qraw = p2.tile([P, NF], F32, name="qraw", tag="qraw",
                                   bufs=2)
                    nc.scalar.activation(qraw, acc, AF.Copy)
                    qswp = p2.tile([P, NF], F32, name="qswp", tag="qswp",
                                   bufs=2)
                    h32 = HD // 2
                    for blk in range(4):
                        src = blk ^ 1
                        nc.sync.dma_start(
                            out=qswp[blk * h32:(blk + 1) * h32, :],
                            in_=qraw[src * h32:(src + 1) * h32, :])
                    tcos = p2.tile([P, NF], F32, name="tcos", tag="tcos",
                                   bufs=2)
                    tsin = p2.tile([P, NF], F32, name="tsin", tag="tsin",
                                   bufs=2)
                    nc.gpsimd.tensor_mul(tcos, qraw, c128)
                    nc.vector.tensor_mul(tsin, qswp, s128)
                    if m < NQT:
                        nc.vector.tensor_add(qT[m][:, ncol], tcos, tsin)
                    else:
                        # k: write rope'd halves into the zero-padded tiles
                        kc = p2.tile([P, NF], BF16, name="kc", tag="kc",
                                     bufs=2)
                        nc.vector.tensor_add(kc, tcos, tsin)
                        nc.vector.tensor_copy(kTp[0][0:HD, ncol],
                                              kc[0:HD, :])
                        nc.vector.tensor_copy(kTp[1][HD:P, ncol],
                                              kc[HD:P, :])

                # v directly in [key, dim] orientation
                def emit_v(kc_i):
                    vacc = pacc.tile([P, P], F32, name="vacc", tag="acc")
                    for k in range(D_TILES):
                        nc.tensor.matmul(
                            vacc,
                            xs[:, k * NF + kc_i * P:k * NF + (kc_i + 1) * P],
                            wv_sb[k], start=(k == 0),
                            stop=(k == D_TILES - 1))
                        yield
                    c = n * (NF // P) + kc_i
                    nc.scalar.activation(v1[c][:, 0:HD], vacc[:, 0:HD],
                                         AF.Copy)
                    nc.scalar.activation(v1[c][:, HD + 1:2 * HD + 1],
                                         vacc[:, HD:2 * HD], AF.Copy)

                # order: k, v, then q0..q3 -- kTp/v1 gate the NEXT qi block's
                # first scores/PV, so they must land well before the boundary
                yield from emit_qk(NQT)
                for kc_i in range(NF // P):
                    yield from emit_v(kc_i)
                for m in range(NQT):
                    yield from emit_qk(m)

            def pull(gen, k):
                if gen is None:
                    return
                for _ in range(k):
                    try:
                        next(gen)
                    except StopIteration:
                        return

            # ---- attention for one (t, qi): 1-step-lag PV pipeline ----
            recb_init = [0]

            def att_block(ts, qi, feed, quota):
                """Attention for pairs `ts` at query block qi, kt-interleaved.
                Two pairs give the PE enough independent work to stay dense
                when there is no projection feed (qi=3)."""
                nk = (qi + 1) * NF // P
                otps = {}
                for i, t in enumerate(ts):
                    for h in (t, t + NQT):
                        pool, tag = (pot, "otp") if i == 0 else (pacc, "acc")
                        otps[h] = pool.tile([HD + 1, NF], F32, name="otp",
                                            tag=tag)
                ats = {}          # (t, kt % 2) -> {h: (at, y0)}
                for kt in range(nk):
                    diag = kt >= qi * NF // P
                    j = kt - qi * NF // P if diag else 0
                    y0 = j * P if diag else 0
                    for t in ts:
                        # both heads' scores into one 2-bank psum tile ->
                        # ONE batched EXP (halves the scalar fixed cost);
                        # the [NF, NF+y0) gap is exp'd but never read
                        sc2 = psc.tile([P, 2 * NF], F32, name="sc2",
                                       tag="sc")
                        for hi in range(2):
                            nc.tensor.matmul(
                                sc2[:, hi * NF + y0:(hi + 1) * NF],
                                kTp[hi][:, kt * P:(kt + 1) * P],
                                qT[t][:, qi * NF + y0:(qi + 1) * NF],
                                start=True, stop=True)
                            pull(feed, quota)
                        at2 = p1.tile([P, 2 * NF], BF16, name="at2",
                                      tag="at", bufs=4)
                        nc.scalar.activation(at2[:, y0:], sc2[:, y0:],
                                             AF.Exp,
                                             scale=float(HD) ** -0.5)
                        if diag:
                            for hi in range(2):
                                nc.vector.tensor_mul(
                                    at2[:, hi * NF + y0:(hi + 1) * NF],
                                    at2[:, hi * NF + y0:(hi + 1) * NF],
                                    cmask[j // 2][:, (j % 2) * NF + y0:
                                                  (j % 2 + 1) * NF])
                        ats[(t, kt % 2)] = (at2, y0)
                    if kt >= 1:
                        for t in ts:
                            at2, py0 = ats[(t, (kt - 1) % 2)]
                            for hi, h in enumerate((t, t + NQT)):
                                nc.tensor.matmul(
                                    otps[h][:, py0:],
                                    v1[kt - 1][:, hi * (HD + 1):
                                               (hi + 1) * (HD + 1)],
                                    at2[:, hi * NF + py0:(hi + 1) * NF],
                                    start=(kt - 1 == 0), stop=False)
                for t in ts:
                    at2, py0 = ats[(t, (nk - 1) % 2)]
                    for hi, h in enumerate((t, t + NQT)):
                        nc.tensor.matmul(
                            otps[h][:, py0:],
                            v1[nk - 1][:, hi * (HD + 1):(hi + 1) * (HD + 1)],
                            at2[:, hi * NF + py0:(hi + 1) * NF],
                            start=(nk - 1 == 0), stop=True)
                # normalization: 1/denom -> broadcast matmul -> scale
                for t in ts:
                    for hi, h in enumerate((t, t + NQT)):
                        otp = otps[h]
                        rec = p1.tile([HD + 1, NF], F32, name="rec",
                                      tag="rec", bufs=2)
                        # full-tile op: the custom DVE op mis-handles
                        # partition-offset slices; rows 0-63 are junk
                        nc.vector.reciprocal_approx_fast(out=rec, in_=otp)
                        recb = p1.tile([P, NF], BF16, name="recb",
                                       tag="recb", bufs=2)
                        if recb_init[0] < 2:
                            # zero rows != 64 once per ring buffer so the
                            # broadcast matmul sees finite values
                            recb_init[0] += 1
                            nc.vector.memset(recb, 0.0)
                        nc.scalar.activation(recb[HD:HD + 1, :],
                                             rec[HD:HD + 1, :], AF.Copy)
                        # broadcast via a full 128x128-mode matmul (no PE
                        # mode switch): lhsT row 64 all-ones, rest zero
                        bc = psc.tile([P, NF], F32, name="bc", tag="sc")
                        nc.tensor.matmul(bc, ones_bc, recb,
                                         start=True, stop=True)
                        # DVE may read only ONE psum operand: stage in SBUF
                        bcs = p1.tile([HD, NF], F32, name="bcs", tag="bcs",
                                      bufs=2)
                        nc.vector.tensor_copy(bcs, bc[0:HD, :])
                        # deep ring: ao frees via its DRAM write, and the
                        # model DMA rings freeze during collectives
                        ao = p1.tile([HD, NF], BF16, name="ao", tag="ao",
                                     bufs=12)
                        nc.vector.tensor_mul(ao, otp[0:HD, :], bcs)
                        nc.sync.dma_start(
                            out=ag_in[t][(qi // 2) * P + hi * HD:
                                         (qi // 2) * P + (hi + 1) * HD,
                                         (qi % 2) * NF:(qi % 2 + 1) * NF],
                            in_=ao)

            def gather(t, h):
                nc.gpsimd.collective_compute(
                    "AllGather", mybir.AluOpType.bypass,
                    replica_groups=groups,
                    ins=[ag_in[t][h * P:(h + 1) * P, :].opt()],
                    outs=[ag_out[t][h * GROUP * P:(h + 1) * GROUP * P,
                                    :].opt()])

            # ---- phase schedule ----
            for _ in proj_chunk(0):
                pass
            for qi in range(N_CHUNKS - 1):
                if qi == 1:
                    drip_wo(0)       # 2MB wo quarters load during qi=1/2,
                if qi == 2:
                    drip_wo(1)       # well before collectives freeze DMA
                feed = proj_chunk(qi + 1)
                n_steps = 2 * GROUP * (qi + 1) * NF // P   # scores MMs
                n_feed = (NQT + 1) * D_TILES + (NF // P) * D_TILES
                quota = (n_feed + n_steps - 1) // n_steps
                for t in range(NQT):
                    att_block([t], qi, feed, quota)
                pull(feed, 10 ** 9)   # flush any remaining proj work
                if qi == 1:
                    for t in range(NQT):
                        gather(t, 0)   # first halves overlap qi=2 attention
            for tp in ((0, 1), (2, 3)):
                att_block(list(tp), N_CHUNKS - 1, None, 0)
                for t in tp:
                    gather(t, 1)

            pot_cm.__exit__(None, None, None)
            psc_cm.__exit__(None, None, None)
            pacc_cm.__exit__(None, None, None)
            p2_cm.__exit__(None, None, None)

            # ---- out projection ----
            with tc.tile_pool(name="oppsum", bufs=8, space="PSUM") as pop, \
                 tc.tile_pool(name="p5", bufs=1) as p5:
                for i in (2, 3):
                    wo_q[i] = p5.tile([P, (WO_R // P) * NF], BF16,
                                      name=f"wo_q{i}")
                    drip_wo(i)
                pid = nc.sync.partition_id()
                rk = pid % GROUP
                # rank r reads gathered half r//2, column block r%2
                row_base = (rk // 2) * (GROUP * P)
                colb = (rk % 2) * NF
                atf = {}
                for t in range(NQT):
                    for g in range(GROUP):
                        a = p5.tile([P, OUT_S], BF16, name=f"atf{t}_{g}")
                        nc.sync.dma_start(
                            out=a,
                            in_=ag_out[t][ds(row_base + g * P, P),
                                          ds(colb, OUT_S)])
                        atf[(t, g)] = a
                kts = [(t, g) for t in range(NQT) for g in range(GROUP)]
                # kidx-outer over 8 accumulators: all t0/t1 contraction work
                # (64 MMs) issues before the first t2 dependency, covering
                # the tail gathers; n_o pairs so wo quarters 2/3 can load
                # during the first wave
                for n_pair in ((0, 1), (2, 3)):
                    ops = {(m, n_o): pop.tile([P, NF], F32, name="op",
                                              tag="op")
                           for m in range(OUT_S // P) for n_o in n_pair}
                    for kidx, (t, g) in enumerate(kts):
                        for m in range(OUT_S // P):
                            for n_o in n_pair:
                                nc.tensor.matmul(
                                    ops[(m, n_o)],
                                    atf[(t, g)][:, m * P:(m + 1) * P],
                                    wo_sb(kidx, n_o),
                                    start=(kidx == 0),
                                    stop=(kidx == len(kts) - 1))
                    for m in range(OUT_S // P):
                        for n_o in n_pair:
                            osb = p1.tile([P, NF], F32, name="osb",
                                          tag="osb", bufs=4)
                            nc.scalar.activation(osb, ops[(m, n_o)], AF.Copy)
                            nc.sync.dma_start(
                                out=out_p[m * P:(m + 1) * P,
                                          n_o * NF:(n_o + 1) * NF],
                                in_=osb)
            p1_cm.__exit__(None, None, None)
            dram_pool.__exit__(None, None, None)
    nc.finalize()
    return nc


# ---------------------------------------------------------------------------
# host-side sharding / unsharding
# ---------------------------------------------------------------------------

def _local_head_perm(nhl):
    nqt = nhl // 2
    order = []
    for t in range(nqt):
        order.append(t)
        order.append(t + nqt)
    return order


def _tile_rows(w):
    """[D_TILES*P, C] -> [P, D_TILES*C]: block k = rows k*P..(k+1)*P."""
    d, c = w.shape
    k = d // P
    return np.ascontiguousarray(
        w.reshape(k, P, c).transpose(1, 0, 2).reshape(P, k * c))


def shard_inputs(x, Wq, Wk, Wv, Wo):
    import ml_dtypes
    dt_ = ml_dtypes.bfloat16
    perm = _local_head_perm(NHL)
    in_maps = []
    nqt = NHL // 2
    row_idx = []
    for t in range(nqt):
        for g in range(GROUP):
            for h in (g * NHL + t, g * NHL + t + nqt):
                row_idx.extend(range(h * HD, (h + 1) * HD))
    # wo: [P, 64*NF]; halves h: blocks i = (n_o - 2h)*16 + kidx
    wo_perm = Wo[row_idx, :].astype(np.float32)  # [2048, 2048]
    wo_blocks = np.zeros((P, (WO_R // P) * WO_C), dtype=dt_)
    i = 0
    for n_o in range(WO_C // NF):
        for kidx in range(WO_R // P):
            wo_blocks[:, i * NF:(i + 1) * NF] = wo_perm[
                kidx * P:(kidx + 1) * P, n_o * NF:(n_o + 1) * NF].astype(dt_)
            i += 1
    for c in range(N_CORES):
        b, rk = c // GROUP, c % GROUP
        col_idx = []
        for t in perm:
            h = rk * NHL + t
            col_idx.extend(range(h * HD, (h + 1) * HD))
        kv_cols = []
        for kvh in range(rk * NKVL, (rk + 1) * NKVL):
            kv_cols.extend(range(kvh * HD, (kvh + 1) * HD))
        # xT tiled: chunk n, block k = x[n*NF:(n+1)*NF, k*P:(k+1)*P].T
        xb = x[b]  # [S, DIM]
        xt = xb.reshape(N_CHUNKS, NF, D_TILES, P).transpose(3, 0, 2, 1)
        # xt[p, n, k, c] = x[n*NF+c, k*P+p] -> [P, N_CHUNKS*D_TILES*NF]
        xt = np.ascontiguousarray(xt.reshape(P, -1)).astype(dt_)
        in_maps.append({
            "xT": xt,
            "wq": _tile_rows(Wq[:, col_idx]).astype(dt_),
            "wk": _tile_rows(Wk[:, kv_cols]).astype(dt_),
            "wv": _tile_rows(Wv[:, kv_cols]).astype(dt_),
            "wo": wo_blocks,
        })
    return in_maps


def unshard_output(results):
    out = np.zeros((B, S, WO_C), dtype=np.float32)
    for c in range(N_CORES):
        b, rk = c // GROUP, c % GROUP
        out[b, rk * OUT_S:(rk + 1) * OUT_S, :] = results[c]["out"]
    return out


_NC_CACHE = {}


def kernel(x, mask=None, Wq=None, Wk=None, Wv=None, Wo=None):
    """Full-input entry point: returns [B, S, DIM] float32."""
    global LAST_RESULTS
    from concourse.bass_utils import run_bass_kernel_spmd

    x = np.asarray(x, dtype=np.float32)
    if "v2" not in _NC_CACHE:
        _NC_CACHE["v2"] = build_nc_v2()
    nc = _NC_CACHE["v2"]
    in_maps = shard_inputs(x, np.asarray(Wq), np.asarray(Wk),
                           np.asarray(Wv), np.asarray(Wo))
    res = run_bass_kernel_spmd(nc, in_maps, core_ids=list(range(N_CORES)),
                               trace=bool(os.environ.get("KERNEL_TRACE")))
    LAST_RESULTS = res
    return unshard_output(res.results)


# revision 16
# speedup vs baseline: 1.0798x; 1.0798x over previous
"""Distributed GQA attention kernel for 8 TRN2 NeuronCores — v2.

Problem: B=2, S=2048, DIM=2048, NH=32 q heads, NKV=8 kv heads, HD=64,
RoPE (base 10000), causal mask, out-projection.

Sharding (8 cores): core c -> batch b = c//4, rank r = c%4.
Each core: 8 q heads (as 4 pair-tiles), 2 kv heads, full S. Host
reassembles out[b, r*512:(r+1)*512, :] from core 4*b + r.

v2 structure (vs v1):
  - Host supplies x TRANSPOSED (xT [DIM, S] bf16): no on-chip transposes.
  - v projected directly in [key, dim] orientation (no vT transpose).
  - Scores matmuls run in 128x128 array mode with zero-padded kT copies
    (kTA/kTB) -> no PE tiling-mode switches, uniform warm matmul stream.
  - qi-outer attention interleaved with the NEXT s-chunk's projections:
    proj matmuls feed the PE queue between scores and PV so the PE never
    waits on the exp chain and HAM stays at full clock.
  - Softmax denominators: ones-column in v1 (PV computes sums), then
    reciprocal_approx_fast (DVE) + K=1 broadcast matmul (no DRAM hop).
  - Causal mask multiplies on DVE (gpsimd is reserved for collectives so
    they can't block compute).
  - Per-pair AllGather of normalized A^T overlapped with later pairs.
"""

import os
import numpy as np

import concourse.bass as bass
import concourse.mybir as mybir
from concourse import bacc, tile
from concourse.bass import ds

F32 = mybir.dt.float32
BF16 = mybir.dt.bfloat16
AF = mybir.ActivationFunctionType

# -------- problem constants (full size) --------
B, S, DIM = 2, 2048, 2048
NH, NKV, HD = 32, 8, 64
ROPE_BASE = 10000.0
N_CORES = 8
GROUP = 4                      # cores per batch group
NHL = NH // GROUP              # 8 local q heads
NKVL = NKV // GROUP            # 2 local kv heads
NQT = NHL // 2                 # 4 head-pair tiles
P = 128                        # partitions
NF = 512                       # free-dim tile (one PSUM bank of f32)
N_CHUNKS = S // NF             # 4 s-chunks
D_TILES = DIM // P             # 16 contraction tiles
KT_TOTAL = S // P              # 16 key tiles
OUT_S = S // GROUP             # 512 output rows per core
WO_R = GROUP * NHL * HD        # 2048
WO_C = DIM

LAST_RESULTS = None


def _rope_tables(S_):
    """cos table tiled to 128 partitions, and a sign-folded sin table:
    rows p with p%64 < 32 carry -sin (x1 half), else +sin (x2 half)."""
    inv_freq = 1.0 / (ROPE_BASE ** (np.arange(0, HD, 2, dtype=np.float64) / HD))
    t = np.arange(S_, dtype=np.float64)
    freqs = inv_freq[:, None] * t[None, :]          # [32, S]
    cos32 = np.cos(freqs).astype(np.float32)
    sin32 = np.sin(freqs).astype(np.float32)
    c128 = np.tile(cos32, (4, 1))                   # [128, S]
    s128 = np.tile(np.concatenate([-sin32, sin32], axis=0), (2, 1))
    return c128, s128


def build_nc_v2():
    import ml_dtypes
    nc = bacc.Bacc(None, target_bir_lowering=False, num_devices=N_CORES)

    # All inputs host-retiled so each SBUF destination loads with ONE big
    # dma_start (fixed cost ~1-2us per DMA makes many small loads brutal):
    #   xt[n][p, k*NF+c]   = x[n*NF+c, k*P+p]      (chunk n, d-tile k)
    #   wq[p, k*DQ+c]      = Wq[k*P+p, c]          etc for wk/wv
    #   wo[h][p, i*NF+c]   = Wo[kidx*P+p, n_o*NF+c], i = (n_o-2h)*16+kidx
    xT_p = nc.declare_dram_parameter("xT", [P, N_CHUNKS * D_TILES * NF], BF16,
                                     isOutput=False)
    wq_p = nc.declare_dram_parameter("wq", [P, D_TILES * NHL * HD], BF16,
                                     isOutput=False)
    wk_p = nc.declare_dram_parameter("wk", [P, D_TILES * NKVL * HD], BF16,
                                     isOutput=False)
    wv_p = nc.declare_dram_parameter("wv", [P, D_TILES * NKVL * HD], BF16,
                                     isOutput=False)
    wo_p = nc.declare_dram_parameter("wo", [P, (WO_R // P) * WO_C], BF16,
                                     isOutput=False)
    out_p = nc.declare_dram_parameter("out", [OUT_S, WO_C], F32, isOutput=True)

    cos_np, sin_np = _rope_tables(S)
    cos_d = nc.inline_tensor(cos_np, name="cos_tab")
    sin_d = nc.inline_tensor(sin_np, name="sin_tab")
    ones_bc_np = np.zeros((P, P), dtype=ml_dtypes.bfloat16)
    ones_bc_np[HD, :] = 1.0
    ones_bc_d = nc.inline_tensor(ones_bc_np, name="ones_bc")
    onesv_d = nc.inline_tensor(np.ones((P, 1), dtype=ml_dtypes.bfloat16),
                               name="ones_col")
    xx = np.arange(P)[:, None]
    yy = np.arange(NF)[None, :]
    mask_np = np.stack([(yy - xx - j * P >= 0) for j in range(NF // P)])
    mask2_np = np.stack([np.concatenate([mask_np[j], mask_np[j + 1]], axis=1)
                         for j in (0, 2)])
    mask_d = nc.inline_tensor(mask2_np.astype(ml_dtypes.bfloat16),
                              name="cmask")

    groups = [list(range(g * GROUP, (g + 1) * GROUP))
              for g in range(N_CORES // GROUP)]

    with tile.TileContext(nc) as tc:
        with tc.tile_pool(name="persist", bufs=1) as pp:
            # comm bounce buffers (DRAM)
            dram_pool = tc.tile_pool(name="dram", bufs=1, space="DRAM")
            dp = dram_pool.__enter__()
            # half-gather layout: ag_in[t] rows h*128.. = qi-half h's A^T
            # (cols (qi%2)*512..); ag_out[t] rows h*512.. = gathered half h
            ag_in = [dp.tile([2 * P, 2 * NF], BF16, name=f"ag_in{t}")
                     for t in range(NQT)]
            ag_out = [dp.tile([2 * GROUP * P, 2 * NF], BF16,
                              name=f"ag_out{t}") for t in range(NQT)]
            dum_in = dp.tile([1, 4], F32, name="dum_in")
            dum_out = dp.tile([GROUP, 4], F32, name="dum_out")

            # dummy gather first: absorbs inter-core launch skew while
            # nothing is pending (collectives freeze the model DMA rings,
            # so real gathers should never be the first sync point)
            nc.gpsimd.collective_compute(
                "AllGather", mybir.AluOpType.bypass, replica_groups=groups,
                ins=[dum_in.opt()], outs=[dum_out.opt()])

            ones_bc = pp.tile([P, P], BF16, name="ones_bc_sb")
            cmask = [pp.tile([P, 2 * NF], BF16, name=f"cmask{j}")
                     for j in range(2)]

            def load_consts():
                nc.sync.dma_start(out=ones_bc, in_=ones_bc_d[:, :])
                for j in range(2):
                    nc.sync.dma_start(out=cmask[j], in_=mask_d[j, :, :])

            qT = [pp.tile([P, S], BF16, name=f"qT{t}") for t in range(NQT)]
            # zero-padded per-head k tiles: kTp[0] has head-A dims on
            # partitions 0-63 (zeros elsewhere), kTp[1] head-B on 64-127
            kTp = [pp.tile([P, S], BF16, name=f"kTp{i}") for i in range(2)]
            nc.vector.memset(kTp[0][HD:P, :], 0.0)
            nc.vector.memset(kTp[1][0:HD, :], 0.0)
            # v1 tiles: [128 keys, 130]: cols 0-63 kv0, 64 ones, 65-128 kv1,
            # 129 ones (ones via memset: no tiny DMAs)
            v1 = [pp.tile([P, 2 * (HD + 1)], BF16, name=f"v1_{c}")
                  for c in range(KT_TOTAL)]
            for c in range(KT_TOTAL):
                nc.vector.memset(v1[c][:, HD:HD + 1], 1.0)
                nc.vector.memset(v1[c][:, 2 * HD + 1:2 * HD + 2], 1.0)

            p1_cm = tc.tile_pool(name="p1", bufs=1)
            p1 = p1_cm.__enter__()
            p2_cm = tc.tile_pool(name="p2", bufs=1)
            p2 = p2_cm.__enter__()
            pacc_cm = tc.tile_pool(name="accpsum", bufs=2, space="PSUM")
            pacc = pacc_cm.__enter__()
            psc_cm = tc.tile_pool(name="scpsum", bufs=2, space="PSUM")
            psc = psc_cm.__enter__()
            pot_cm = tc.tile_pool(name="otpsum", bufs=2, space="PSUM")
            pot = pot_cm.__enter__()

            # ---- input loads: one big dma_start per logical input ----
            # xt chunk tile: [128, 16*512]; slice k = d-tile k's s-chunk
            def load_x_chunk(n):
                xs = p2.tile([P, D_TILES * NF], BF16, name="xch", tag="xch",
                             bufs=2)
                nc.sync.dma_start(
                    out=xs, in_=xT_p[:, n * D_TILES * NF:(n + 1) * D_TILES * NF])
                return xs

            DQ = NHL * HD
            DKV = NKVL * HD
            wq_all = p2.tile([P, D_TILES * DQ], BF16, name="wq_all")
            wk_all = p2.tile([P, D_TILES * DKV], BF16, name="wk_all")
            wv_all = p2.tile([P, D_TILES * DKV], BF16, name="wv_all")
            xsl0 = p2.tile([P, D_TILES * NF], BF16, name="xch", tag="xch",
                           bufs=2)
            # x on the sync ring, weights on the scalar ring: the two HWDGE
            # rings run concurrently, halving time-to-first-matmul
            nc.sync.dma_start(out=xsl0, in_=xT_p[:, 0:D_TILES * NF])
            nc.scalar.dma_start(out=wq_all, in_=wq_p[:, :])
            nc.scalar.dma_start(out=wk_all, in_=wk_p[:, :])
            nc.scalar.dma_start(out=wv_all, in_=wv_p[:, :])
            load_consts()
            wq_sb = [wq_all[:, k * DQ:(k + 1) * DQ] for k in range(D_TILES)]
            wk_sb = [wk_all[:, k * DKV:(k + 1) * DKV] for k in range(D_TILES)]
            wv_sb = [wv_all[:, k * DKV:(k + 1) * DKV] for k in range(D_TILES)]
            xsl = {0: xsl0}

            c_all = p2.tile([P, S], F32, name="c_all")
            s_all = p2.tile([P, S], F32, name="s_all")
            nc.scalar.dma_start(out=c_all, in_=cos_d[:, :])
            nc.scalar.dma_start(out=s_all, in_=sin_d[:, :])

            def load_cs(n):
                return (c_all[:, n * NF:(n + 1) * NF],
                        s_all[:, n * NF:(n + 1) * NF])

            # Wo: four resident 2MB quarters (quarter = one n_o column
            # group). Quarters 0/1 load into p1 during attention; 2/3 load
            # into the out-projection pool after the proj-phase pool frees.
            wo_q = {}
            wo_q[0] = p1.tile([P, (WO_R // P) * NF], BF16, name="wo_q0")
            wo_q[1] = p1.tile([P, (WO_R // P) * NF], BF16, name="wo_q1")

            def wo_sb(kidx, n_o):
                return wo_q[n_o][:, kidx * NF:(kidx + 1) * NF]

            def drip_wo(i):
                nc.sync.dma_start(
                    out=wo_q[i],
                    in_=wo_p[:, i * (WO_R // P) * NF:
                             (i + 1) * (WO_R // P) * NF])

            # ---- projection chunk as a generator: yields after each PE op
            def proj_chunk(n):
                ncol = slice(n * NF, (n + 1) * NF)
                xs = xsl[n]
                if n + 1 < N_CHUNKS:
                    xsl[n + 1] = load_x_chunk(n + 1)  # prefetch
                c128, s128 = load_cs(n)
                # q pair tiles + k, m-outer k-inner (one psum bank each)
                for m in range(NQT + 1):
                    acc = pacc.tile([P, NF], F32, name="acc", tag="acc")
                    for k in range(D_TILES):
                        if m < NQT:
                            lhsT = wq_sb[k][:, m * P:(m + 1) * P]
                        else:
                            lhsT = wk_sb[k]
                        nc.tensor.matmul(acc, lhsT,
                                         xs[:, k * NF:(k + 1) * NF],
                                         start=(k == 0),
                                         stop=(k == D_TILES - 1))
                        yield
                    # rope: out = raw*cos + swap(raw)*sgn_sin
                    qraw = p2.tile([P, NF], F32, name="qraw", tag="qraw",
                                   bufs=2)
                    nc.scalar.activation(qraw, acc, AF.Copy)
                    qswp = p2.tile([P, NF], F32, name="qswp", tag="qswp",
                                   bufs=2)
                    h32 = HD // 2
                    for blk in range(4):
                        src = blk ^ 1
                        nc.sync.dma_start(
                            out=qswp[blk * h32:(blk + 1) * h32, :],
                            in_=qraw[src * h32:(src + 1) * h32, :])
                    tcos = p2.tile([P, NF], F32, name="tcos", tag="tcos",
                                   bufs=2)
                    tsin = p2.tile([P, NF], F32, name="tsin", tag="tsin",
                                   bufs=2)
                    nc.gpsimd.tensor_mul(tcos, qraw, c128)
                    nc.vector.tensor_mul(tsin, qswp, s128)
                    if m < NQT:
                        nc.vector.tensor_add(qT[m][:, ncol], tcos, tsin)
                    else:
                        # k: write rope'd halves into the zero-padded tiles
                        kc = p2.tile([P, NF], BF16, name="kc", tag="kc",
                                     bufs=2)
                        nc.vector.tensor_add(kc, tcos, tsin)
                        nc.vector.tensor_copy(kTp[0][0:HD, ncol],
                                              kc[0:HD, :])
                        nc.vector.tensor_copy(kTp[1][HD:P, ncol],
                                              kc[HD:P, :])
                # v directly in [key, dim] orientation
                for kc_i in range(NF // P):
                    vacc = pacc.tile([P, P], F32, name="vacc", tag="acc")
                    for k in range(D_TILES):
                        nc.tensor.matmul(
                            vacc,
                            xs[:, k * NF + kc_i * P:k * NF + (kc_i + 1) * P],
                            wv_sb[k], start=(k == 0),
                            stop=(k == D_TILES - 1))
                        yield
                    c = n * (NF // P) + kc_i
                    nc.scalar.activation(v1[c][:, 0:HD], vacc[:, 0:HD],
                                         AF.Copy)
                    nc.scalar.activation(v1[c][:, HD + 1:2 * HD + 1],
                                         vacc[:, HD:2 * HD], AF.Copy)

            def pull(gen, k):
                if gen is None:
                    return
                for _ in range(k):
                    try:
                        next(gen)
                    except StopIteration:
                        return

            # ---- attention for one (t, qi): 1-step-lag PV pipeline ----
            recb_init = [0]

            def att_block(ts, qi, feed, quota):
                """Attention for pairs `ts` at query block qi, kt-interleaved.
                Two pairs give the PE enough independent work to stay dense
                when there is no projection feed (qi=3)."""
                nk = (qi + 1) * NF // P
                otps = {}
                for i, t in enumerate(ts):
                    for h in (t, t + NQT):
                        pool, tag = (pot, "otp") if i == 0 else (pacc, "acc")
                        otps[h] = pool.tile([HD + 1, NF], F32, name="otp",
                                            tag=tag)
                ats = {}          # (t, kt % 2) -> {h: (at, y0)}
                for kt in range(nk):
                    diag = kt >= qi * NF // P
                    j = kt - qi * NF // P if diag else 0
                    y0 = j * P if diag else 0
                    for t in ts:
                        # both heads' scores into one 2-bank psum tile ->
                        # ONE batched EXP (halves the scalar fixed cost);
                        # the [NF, NF+y0) gap is exp'd but never read
                        sc2 = psc.tile([P, 2 * NF], F32, name="sc2",
                                       tag="sc")
                        for hi in range(2):
                            nc.tensor.matmul(
                                sc2[:, hi * NF + y0:(hi + 1) * NF],
                                kTp[hi][:, kt * P:(kt + 1) * P],
                                qT[t][:, qi * NF + y0:(qi + 1) * NF],
                                start=True, stop=True)
                            pull(feed, quota)
                        at2 = p1.tile([P, 2 * NF], BF16, name="at2",
                                      tag="at", bufs=4)
                        nc.scalar.activation(at2[:, y0:], sc2[:, y0:],
                                             AF.Exp,
                                             scale=float(HD) ** -0.5)
                        if diag:
                            for hi in range(2):
                                nc.vector.tensor_mul(
                                    at2[:, hi * NF + y0:(hi + 1) * NF],
                                    at2[:, hi * NF + y0:(hi + 1) * NF],
                                    cmask[j // 2][:, (j % 2) * NF + y0:
                                                  (j % 2 + 1) * NF])
                        ats[(t, kt % 2)] = (at2, y0)
                    if kt >= 1:
                        for t in ts:
                            at2, py0 = ats[(t, (kt - 1) % 2)]
                            for hi, h in enumerate((t, t + NQT)):
                                nc.tensor.matmul(
                                    otps[h][:, py0:],
                                    v1[kt - 1][:, hi * (HD + 1):
                                               (hi + 1) * (HD + 1)],
                                    at2[:, hi * NF + py0:(hi + 1) * NF],
                                    start=(kt - 1 == 0), stop=False)
                for t in ts:
                    at2, py0 = ats[(t, (nk - 1) % 2)]
                    for hi, h in enumerate((t, t + NQT)):
                        nc.tensor.matmul(
                            otps[h][:, py0:],
                            v1[nk - 1][:, hi * (HD + 1):(hi + 1) * (HD + 1)],
                            at2[:, hi * NF + py0:(hi + 1) * NF],
                            start=(nk - 1 == 0), stop=True)
                # normalization: 1/denom -> broadcast matmul -> scale
                for t in ts:
                    for hi, h in enumerate((t, t + NQT)):
                        otp = otps[h]
                        rec = p1.tile([HD + 1, NF], F32, name="rec",
                                      tag="rec", bufs=2)
                        # full-tile op: the custom DVE op mis-handles
                        # partition-offset slices; rows 0-63 are junk
                        nc.vector.reciprocal_approx_fast(out=rec, in_=otp)
                        recb = p1.tile([P, NF], BF16, name="recb",
                                       tag="recb", bufs=2)
                        if recb_init[0] < 2:
                            # zero rows != 64 once per ring buffer so the
                            # broadcast matmul sees finite values
                            recb_init[0] += 1
                            nc.vector.memset(recb, 0.0)
                        nc.scalar.activation(recb[HD:HD + 1, :],
                                             rec[HD:HD + 1, :], AF.Copy)
                        # broadcast via a full 128x128-mode matmul (no PE
                        # mode switch): lhsT row 64 all-ones, rest zero
                        bc = psc.tile([P, NF], F32, name="bc", tag="sc")
                        nc.tensor.matmul(bc, ones_bc, recb,
                                         start=True, stop=True)
                        # DVE may read only ONE psum operand: stage in SBUF
                        bcs = p1.tile([HD, NF], F32, name="bcs", tag="bcs",
                                      bufs=2)
                        nc.vector.tensor_copy(bcs, bc[0:HD, :])
                        # deep ring: ao frees via its DRAM write, and the
                        # model DMA rings freeze during collectives
                        ao = p1.tile([HD, NF], BF16, name="ao", tag="ao",
                                     bufs=12)
                        nc.vector.tensor_mul(ao, otp[0:HD, :], bcs)
                        nc.sync.dma_start(
                            out=ag_in[t][(qi // 2) * P + hi * HD:
                                         (qi // 2) * P + (hi + 1) * HD,
                                         (qi % 2) * NF:(qi % 2 + 1) * NF],
                            in_=ao)

            def gather(t, h):
                nc.gpsimd.collective_compute(
                    "AllGather", mybir.AluOpType.bypass,
                    replica_groups=groups,
                    ins=[ag_in[t][h * P:(h + 1) * P, :].opt()],
                    outs=[ag_out[t][h * GROUP * P:(h + 1) * GROUP * P,
                                    :].opt()])

            # ---- phase schedule ----
            for _ in proj_chunk(0):
                pass
            for qi in range(N_CHUNKS - 1):
                if qi == 1:
                    drip_wo(0)       # 2MB wo quarters load during qi=1/2,
                if qi == 2:
                    drip_wo(1)       # well before collectives freeze DMA
                feed = proj_chunk(qi + 1)
                n_steps = 2 * GROUP * (qi + 1) * NF // P   # scores MMs
                n_feed = (NQT + 1) * D_TILES + (NF // P) * D_TILES
                quota = (n_feed + n_steps - 1) // n_steps
                for t in range(NQT):
                    att_block([t], qi, feed, quota)
                pull(feed, 10 ** 9)   # flush any remaining proj work
                if qi == 1:
                    for t in range(NQT):
                        gather(t, 0)   # first halves overlap qi=2 attention
            for tp in ((0, 1), (2, 3)):
                att_block(list(tp), N_CHUNKS - 1, None, 0)
                for t in tp:
                    gather(t, 1)

            pot_cm.__exit__(None, None, None)
            psc_cm.__exit__(None, None, None)
            pacc_cm.__exit__(None, None, None)
            p2_cm.__exit__(None, None, None)

            # ---- out projection ----
            with tc.tile_pool(name="oppsum", bufs=8, space="PSUM") as pop, \
                 tc.tile_pool(name="p5", bufs=1) as p5:
                for i in (2, 3):
                    wo_q[i] = p5.tile([P, (WO_R // P) * NF], BF16,
                                      name=f"wo_q{i}")
                    drip_wo(i)
                pid = nc.sync.partition_id()
                rk = pid % GROUP
                # rank r reads gathered half r//2, column block r%2
                row_base = (rk // 2) * (GROUP * P)
                colb = (rk % 2) * NF
                atf = {}
                for t in range(NQT):
                    for g in range(GROUP):
                        a = p5.tile([P, OUT_S], BF16, name=f"atf{t}_{g}")
                        nc.sync.dma_start(
                            out=a,
                            in_=ag_out[t][ds(row_base + g * P, P),
                                          ds(colb, OUT_S)])
                        atf[(t, g)] = a
                kts = [(t, g) for t in range(NQT) for g in range(GROUP)]
                # kidx-outer over 8 accumulators: all t0/t1 contraction work
                # (64 MMs) issues before the first t2 dependency, covering
                # the tail gathers; n_o pairs so wo quarters 2/3 can load
                # during the first wave
                for n_pair in ((0, 1), (2, 3)):
                    ops = {(m, n_o): pop.tile([P, NF], F32, name="op",
                                              tag="op")
                           for m in range(OUT_S // P) for n_o in n_pair}
                    for kidx, (t, g) in enumerate(kts):
                        for m in range(OUT_S // P):
                            for n_o in n_pair:
                                nc.tensor.matmul(
                                    ops[(m, n_o)],
                                    atf[(t, g)][:, m * P:(m + 1) * P],
                                    wo_sb(kidx, n_o),
                                    start=(kidx == 0),
                                    stop=(kidx == len(kts) - 1))
                    for m in range(OUT_S // P):
                        for n_o in n_pair:
                            osb = p1.tile([P, NF], F32, name="osb",
                                          tag="osb", bufs=4)
                            nc.scalar.activation(osb, ops[(m, n_o)], AF.Copy)
                            nc.sync.dma_start(
                                out=out_p[m * P:(m + 1) * P,
                                          n_o * NF:(n_o + 1) * NF],
                                in_=osb)
            p1_cm.__exit__(None, None, None)
            dram_pool.__exit__(None, None, None)
    nc.finalize()
    return nc


# ---------------------------------------------------------------------------
# host-side sharding / unsharding
# ---------------------------------------------------------------------------

def _local_head_perm(nhl):
    nqt = nhl // 2
    order = []
    for t in range(nqt):
        order.append(t)
        order.append(t + nqt)
    return order


def _tile_rows(w):
    """[D_TILES*P, C] -> [P, D_TILES*C]: block k = rows k*P..(k+1)*P."""
    d, c = w.shape
    k = d // P
    return np.ascontiguousarray(
        w.reshape(k, P, c).transpose(1, 0, 2).reshape(P, k * c))


def shard_inputs(x, Wq, Wk, Wv, Wo):
    import ml_dtypes
    dt_ = ml_dtypes.bfloat16
    perm = _local_head_perm(NHL)
    in_maps = []
    nqt = NHL // 2
    row_idx = []
    for t in range(nqt):
        for g in range(GROUP):
            for h in (g * NHL + t, g * NHL + t + nqt):
                row_idx.extend(range(h * HD, (h + 1) * HD))
    # wo: [P, 64*NF]; halves h: blocks i = (n_o - 2h)*16 + kidx
    wo_perm = Wo[row_idx, :].astype(np.float32)  # [2048, 2048]
    wo_blocks = np.zeros((P, (WO_R // P) * WO_C), dtype=dt_)
    i = 0
    for n_o in range(WO_C // NF):
        for kidx in range(WO_R // P):
            wo_blocks[:, i * NF:(i + 1) * NF] = wo_perm[
                kidx * P:(kidx + 1) * P, n_o * NF:(n_o + 1) * NF].astype(dt_)
            i += 1
    for c in range(N_CORES):
        b, rk = c // GROUP, c % GROUP
        col_idx = []
        for t in perm:
            h = rk * NHL + t
            col_idx.extend(range(h * HD, (h + 1) * HD))
        kv_cols = []
        for kvh in range(rk * NKVL, (rk + 1) * NKVL):
            kv_cols.extend(range(kvh * HD, (kvh + 1) * HD))
        # xT tiled: chunk n, block k = x[n*NF:(n+1)*NF, k*P:(k+1)*P].T
        xb = x[b]  # [S, DIM]
        xt = xb.reshape(N_CHUNKS, NF, D_TILES, P).transpose(3, 0, 2, 1)
        # xt[p, n, k, c] = x[n*NF+c, k*P+p] -> [P, N_CHUNKS*D_TILES*NF]
        xt = np.ascontiguousarray(xt.reshape(P, -1)).astype(dt_)
        in_maps.append({
            "xT": xt,
            "wq": _tile_rows(Wq[:, col_idx]).astype(dt_),
            "wk": _tile_rows(Wk[:, kv_cols]).astype(dt_),
            "wv": _tile_rows(Wv[:, kv_cols]).astype(dt_),
            "wo": wo_blocks,
        })
    return in_maps


def unshard_output(results):
    out = np.zeros((B, S, WO_C), dtype=np.float32)
    for c in range(N_CORES):
        b, rk = c // GROUP, c % GROUP
        out[b, rk * OUT_S:(rk + 1) * OUT_S, :] = results[c]["out"]
    return out


_NC_CACHE = {}


def kernel(x, mask=None, Wq=None, Wk=None, Wv=None, Wo=None):
    """Full-input entry point: returns [B, S, DIM] float32."""
    global LAST_RESULTS
    from concourse.bass_utils import run_bass_kernel_spmd

    x = np.asarray(x, dtype=np.float32)
    if "v2" not in _NC_CACHE:
        _NC_CACHE["v2"] = build_nc_v2()
    nc = _NC_CACHE["v2"]
    in_maps = shard_inputs(x, np.asarray(Wq), np.asarray(Wk),
                           np.asarray(Wv), np.asarray(Wo))
    res = run_bass_kernel_spmd(nc, in_maps, core_ids=list(range(N_CORES)),
                               trace=bool(os.environ.get("KERNEL_TRACE")))
    LAST_RESULTS = res
    return unshard_output(res.results)


# revision 17
# speedup vs baseline: 1.1173x; 1.0348x over previous
"""Distributed GQA attention kernel for 8 TRN2 NeuronCores — v2.

Problem: B=2, S=2048, DIM=2048, NH=32 q heads, NKV=8 kv heads, HD=64,
RoPE (base 10000), causal mask, out-projection.

Sharding (8 cores): core c -> batch b = c//4, rank r = c%4.
Each core: 8 q heads (as 4 pair-tiles), 2 kv heads, full S. Host
reassembles out[b, r*512:(r+1)*512, :] from core 4*b + r.

v2 structure (vs v1):
  - Host supplies x TRANSPOSED (xT [DIM, S] bf16): no on-chip transposes.
  - v projected directly in [key, dim] orientation (no vT transpose).
  - Scores matmuls run in 128x128 array mode with zero-padded kT copies
    (kTA/kTB) -> no PE tiling-mode switches, uniform warm matmul stream.
  - qi-outer attention interleaved with the NEXT s-chunk's projections:
    proj matmuls feed the PE queue between scores and PV so the PE never
    waits on the exp chain and HAM stays at full clock.
  - Softmax denominators: ones-column in v1 (PV computes sums), then
    reciprocal_approx_fast (DVE) + K=1 broadcast matmul (no DRAM hop).
  - Causal mask multiplies on DVE (gpsimd is reserved for collectives so
    they can't block compute).
  - Per-pair AllGather of normalized A^T overlapped with later pairs.
"""

import os
import numpy as np

import concourse.bass as bass
import concourse.mybir as mybir
from concourse import bacc, tile
from concourse.bass import ds

F32 = mybir.dt.float32
BF16 = mybir.dt.bfloat16
AF = mybir.ActivationFunctionType

# -------- problem constants (full size) --------
B, S, DIM = 2, 2048, 2048
NH, NKV, HD = 32, 8, 64
ROPE_BASE = 10000.0
N_CORES = 8
GROUP = 4                      # cores per batch group
NHL = NH // GROUP              # 8 local q heads
NKVL = NKV // GROUP            # 2 local kv heads
NQT = NHL // 2                 # 4 head-pair tiles
P = 128                        # partitions
NF = 512                       # free-dim tile (one PSUM bank of f32)
N_CHUNKS = S // NF             # 4 s-chunks
D_TILES = DIM // P             # 16 contraction tiles
KT_TOTAL = S // P              # 16 key tiles
OUT_S = S // GROUP             # 512 output rows per core
WO_R = GROUP * NHL * HD        # 2048
WO_C = DIM

LAST_RESULTS = None


def _rope_tables(S_):
    """cos table tiled to 128 partitions, and a sign-folded sin table:
    rows p with p%64 < 32 carry -sin (x1 half), else +sin (x2 half)."""
    inv_freq = 1.0 / (ROPE_BASE ** (np.arange(0, HD, 2, dtype=np.float64) / HD))
    t = np.arange(S_, dtype=np.float64)
    freqs = inv_freq[:, None] * t[None, :]          # [32, S]
    cos32 = np.cos(freqs).astype(np.float32)
    sin32 = np.sin(freqs).astype(np.float32)
    c128 = np.tile(cos32, (4, 1))                   # [128, S]
    s128 = np.tile(np.concatenate([-sin32, sin32], axis=0), (2, 1))
    return c128, s128


def build_nc_v2():
    import ml_dtypes
    nc = bacc.Bacc(None, target_bir_lowering=False, num_devices=N_CORES)

    # All inputs host-retiled so each SBUF destination loads with ONE big
    # dma_start (fixed cost ~1-2us per DMA makes many small loads brutal):
    #   xt[n][p, k*NF+c]   = x[n*NF+c, k*P+p]      (chunk n, d-tile k)
    #   wq[p, k*DQ+c]      = Wq[k*P+p, c]          etc for wk/wv
    #   wo[h][p, i*NF+c]   = Wo[kidx*P+p, n_o*NF+c], i = (n_o-2h)*16+kidx
    xT_p = nc.declare_dram_parameter("xT", [P, N_CHUNKS * D_TILES * NF], BF16,
                                     isOutput=False)
    wq_p = nc.declare_dram_parameter("wq", [P, D_TILES * NHL * HD], BF16,
                                     isOutput=False)
    wk_p = nc.declare_dram_parameter("wk", [P, D_TILES * NKVL * HD], BF16,
                                     isOutput=False)
    wv_p = nc.declare_dram_parameter("wv", [P, D_TILES * NKVL * HD], BF16,
                                     isOutput=False)
    wo_p = nc.declare_dram_parameter("wo", [P, (WO_R // P) * WO_C], BF16,
                                     isOutput=False)
    out_p = nc.declare_dram_parameter("out", [OUT_S, WO_C], F32, isOutput=True)

    cos_np, sin_np = _rope_tables(S)
    cos_d = nc.inline_tensor(cos_np, name="cos_tab")
    sin_d = nc.inline_tensor(sin_np, name="sin_tab")
    ones_bc_np = np.zeros((P, P), dtype=ml_dtypes.bfloat16)
    ones_bc_np[HD, :] = 1.0
    ones_bc_d = nc.inline_tensor(ones_bc_np, name="ones_bc")
    onesv_d = nc.inline_tensor(np.ones((P, 1), dtype=ml_dtypes.bfloat16),
                               name="ones_col")
    xx = np.arange(P)[:, None]
    yy = np.arange(NF)[None, :]
    mask_np = np.stack([(yy - xx - j * P >= 0) for j in range(NF // P)])
    mask2_np = np.stack([np.concatenate([mask_np[j], mask_np[j + 1]], axis=1)
                         for j in (0, 2)])
    mask_d = nc.inline_tensor(mask2_np.astype(ml_dtypes.bfloat16),
                              name="cmask")

    groups = [list(range(g * GROUP, (g + 1) * GROUP))
              for g in range(N_CORES // GROUP)]

    with tile.TileContext(nc) as tc:
        with tc.tile_pool(name="persist", bufs=1) as pp:
            # comm bounce buffers (DRAM)
            dram_pool = tc.tile_pool(name="dram", bufs=1, space="DRAM")
            dp = dram_pool.__enter__()
            # half-gather layout: ag_in[t] rows h*128.. = qi-half h's A^T
            # (cols (qi%2)*512..); ag_out[t] rows h*512.. = gathered half h
            ag_in = [dp.tile([2 * P, 2 * NF], BF16, name=f"ag_in{t}")
                     for t in range(NQT)]
            ag_out = [dp.tile([2 * GROUP * P, 2 * NF], BF16,
                              name=f"ag_out{t}") for t in range(NQT)]
            dum_in = dp.tile([1, 4], F32, name="dum_in")
            dum_out = dp.tile([GROUP, 4], F32, name="dum_out")

            # dummy gather first: absorbs inter-core launch skew while
            # nothing is pending (collectives freeze the model DMA rings,
            # so real gathers should never be the first sync point)
            nc.gpsimd.collective_compute(
                "AllGather", mybir.AluOpType.bypass, replica_groups=groups,
                ins=[dum_in.opt()], outs=[dum_out.opt()])

            ones_bc = pp.tile([P, P], BF16, name="ones_bc_sb")
            cmask = [pp.tile([P, 2 * NF], BF16, name=f"cmask{j}")
                     for j in range(2)]

            def load_consts():
                nc.sync.dma_start(out=ones_bc, in_=ones_bc_d[:, :])
                for j in range(2):
                    nc.sync.dma_start(out=cmask[j], in_=mask_d[j, :, :])

            qT = [pp.tile([P, S], BF16, name=f"qT{t}") for t in range(NQT)]
            # zero-padded per-head k tiles: kTp[0] has head-A dims on
            # partitions 0-63 (zeros elsewhere), kTp[1] head-B on 64-127
            kTp = [pp.tile([P, S], BF16, name=f"kTp{i}") for i in range(2)]
            nc.vector.memset(kTp[0][HD:P, :], 0.0)
            nc.vector.memset(kTp[1][0:HD, :], 0.0)
            # v1 tiles: [128 keys, 130]: cols 0-63 kv0, 64 ones, 65-128 kv1,
            # 129 ones (ones via memset: no tiny DMAs)
            v1 = [pp.tile([P, 2 * (HD + 1)], BF16, name=f"v1_{c}")
                  for c in range(KT_TOTAL)]
            for c in range(KT_TOTAL):
                nc.vector.memset(v1[c][:, HD:HD + 1], 1.0)
                nc.vector.memset(v1[c][:, 2 * HD + 1:2 * HD + 2], 1.0)

            p1_cm = tc.tile_pool(name="p1", bufs=1)
            p1 = p1_cm.__enter__()
            p2_cm = tc.tile_pool(name="p2", bufs=1)
            p2 = p2_cm.__enter__()
            pacc_cm = tc.tile_pool(name="accpsum", bufs=2, space="PSUM")
            pacc = pacc_cm.__enter__()
            psc_cm = tc.tile_pool(name="scpsum", bufs=2, space="PSUM")
            psc = psc_cm.__enter__()
            pot_cm = tc.tile_pool(name="otpsum", bufs=2, space="PSUM")
            pot = pot_cm.__enter__()

            # ---- input loads: one big dma_start per logical input ----
            # xt chunk tile: [128, 16*512]; slice k = d-tile k's s-chunk
            def load_x_chunk(n):
                xs = p2.tile([P, D_TILES * NF], BF16, name="xch", tag="xch",
                             bufs=2)
                nc.sync.dma_start(
                    out=xs, in_=xT_p[:, n * D_TILES * NF:(n + 1) * D_TILES * NF])
                return xs

            DQ = NHL * HD
            DKV = NKVL * HD
            wq_all = p2.tile([P, D_TILES * DQ], BF16, name="wq_all")
            wk_all = p2.tile([P, D_TILES * DKV], BF16, name="wk_all")
            wv_all = p2.tile([P, D_TILES * DKV], BF16, name="wv_all")
            xsl0 = p2.tile([P, D_TILES * NF], BF16, name="xch", tag="xch",
                           bufs=2)
            # x on the sync ring, weights on the scalar ring: the two HWDGE
            # rings run concurrently, halving time-to-first-matmul
            nc.sync.dma_start(out=xsl0, in_=xT_p[:, 0:D_TILES * NF])
            nc.scalar.dma_start(out=wq_all, in_=wq_p[:, :])
            nc.scalar.dma_start(out=wk_all, in_=wk_p[:, :])
            nc.scalar.dma_start(out=wv_all, in_=wv_p[:, :])
            load_consts()
            wq_sb = [wq_all[:, k * DQ:(k + 1) * DQ] for k in range(D_TILES)]
            wk_sb = [wk_all[:, k * DKV:(k + 1) * DKV] for k in range(D_TILES)]
            wv_sb = [wv_all[:, k * DKV:(k + 1) * DKV] for k in range(D_TILES)]
            xsl = {0: xsl0}

            c_all = p2.tile([P, S], F32, name="c_all")
            s_all = p2.tile([P, S], F32, name="s_all")
            nc.scalar.dma_start(out=c_all, in_=cos_d[:, :])
            nc.scalar.dma_start(out=s_all, in_=sin_d[:, :])

            def load_cs(n):
                return (c_all[:, n * NF:(n + 1) * NF],
                        s_all[:, n * NF:(n + 1) * NF])

            # Wo: four resident 2MB quarters (quarter = one n_o column
            # group). Quarters 0/1 load into p1 during attention; 2/3 load
            # into the out-projection pool after the proj-phase pool frees.
            wo_q = {}
            wo_q[0] = p1.tile([P, (WO_R // P) * NF], BF16, name="wo_q0")
            wo_q[1] = p1.tile([P, (WO_R // P) * NF], BF16, name="wo_q1")

            def wo_sb(kidx, n_o):
                return wo_q[n_o][:, kidx * NF:(kidx + 1) * NF]

            def drip_wo(i):
                nc.sync.dma_start(
                    out=wo_q[i],
                    in_=wo_p[:, i * (WO_R // P) * NF:
                             (i + 1) * (WO_R // P) * NF])

            # ---- projection chunk as a generator: yields after each PE op
            def proj_chunk(n):
                ncol = slice(n * NF, (n + 1) * NF)
                xs = xsl[n]
                if n + 1 < N_CHUNKS:
                    xsl[n + 1] = load_x_chunk(n + 1)  # prefetch
                c128, s128 = load_cs(n)
                # q pair tiles + k, m-outer k-inner (one psum bank each)
                for m in range(NQT + 1):
                    acc = pacc.tile([P, NF], F32, name="acc", tag="acc")
                    for k in range(D_TILES):
                        if m < NQT:
                            lhsT = wq_sb[k][:, m * P:(m + 1) * P]
                        else:
                            lhsT = wk_sb[k]
                        nc.tensor.matmul(acc, lhsT,
                                         xs[:, k * NF:(k + 1) * NF],
                                         start=(k == 0),
                                         stop=(k == D_TILES - 1))
                        yield
                    # rope: out = raw*cos + swap(raw)*sgn_sin
                    qraw = p2.tile([P, NF], F32, name="qraw", tag="qraw",
                                   bufs=2)
                    nc.scalar.activation(qraw, acc, AF.Copy)
                    qswp = p2.tile([P, NF], F32, name="qswp", tag="qswp",
                                   bufs=2)
                    h32 = HD // 2
                    for blk in range(4):
                        src = blk ^ 1
                        nc.sync.dma_start(
                            out=qswp[blk * h32:(blk + 1) * h32, :],
                            in_=qraw[src * h32:(src + 1) * h32, :])
                    tcos = p2.tile([P, NF], F32, name="tcos", tag="tcos",
                                   bufs=2)
                    tsin = p2.tile([P, NF], F32, name="tsin", tag="tsin",
                                   bufs=2)
                    nc.gpsimd.tensor_mul(tcos, qraw, c128)
                    nc.vector.tensor_mul(tsin, qswp, s128)
                    if m < NQT:
                        nc.vector.tensor_add(qT[m][:, ncol], tcos, tsin)
                    else:
                        # k: write rope'd halves into the zero-padded tiles
                        kc = p2.tile([P, NF], BF16, name="kc", tag="kc",
                                     bufs=2)
                        nc.vector.tensor_add(kc, tcos, tsin)
                        nc.vector.tensor_copy(kTp[0][0:HD, ncol],
                                              kc[0:HD, :])
                        nc.vector.tensor_copy(kTp[1][HD:P, ncol],
                                              kc[HD:P, :])
                # v directly in [key, dim] orientation
                for kc_i in range(NF // P):
                    vacc = pacc.tile([P, P], F32, name="vacc", tag="acc")
                    for k in range(D_TILES):
                        nc.tensor.matmul(
                            vacc,
                            xs[:, k * NF + kc_i * P:k * NF + (kc_i + 1) * P],
                            wv_sb[k], start=(k == 0),
                            stop=(k == D_TILES - 1))
                        yield
                    c = n * (NF // P) + kc_i
                    nc.scalar.activation(v1[c][:, 0:HD], vacc[:, 0:HD],
                                         AF.Copy)
                    nc.scalar.activation(v1[c][:, HD + 1:2 * HD + 1],
                                         vacc[:, HD:2 * HD], AF.Copy)

            def pull(gen, k):
                if gen is None:
                    return
                for _ in range(k):
                    try:
                        next(gen)
                    except StopIteration:
                        return

            # ---- attention for one (t, qi): 1-step-lag PV pipeline ----
            recb_init = [0]

            def att_block(ts, qi, feed, quota):
                """Attention for pairs `ts` at query block qi, kt-interleaved.
                Two pairs give the PE enough independent work to stay dense
                when there is no projection feed (qi=3)."""
                nk = (qi + 1) * NF // P
                otps = {}
                for i, t in enumerate(ts):
                    for h in (t, t + NQT):
                        pool, tag = (pot, "otp") if i == 0 else (pacc, "acc")
                        otps[h] = pool.tile([HD + 1, NF], F32, name="otp",
                                            tag=tag)
                ats = {}          # (t, kt % 2) -> {h: (at, y0)}
                for kt in range(nk):
                    diag = kt >= qi * NF // P
                    j = kt - qi * NF // P if diag else 0
                    y0 = j * P if diag else 0
                    for t in ts:
                        # both heads' scores into one 2-bank psum tile ->
                        # ONE batched EXP (halves the scalar fixed cost);
                        # the [NF, NF+y0) gap is exp'd but never read
                        sc2 = psc.tile([P, 2 * NF], F32, name="sc2",
                                       tag="sc")
                        for hi in range(2):
                            nc.tensor.matmul(
                                sc2[:, hi * NF + y0:(hi + 1) * NF],
                                kTp[hi][:, kt * P:(kt + 1) * P],
                                qT[t][:, qi * NF + y0:(qi + 1) * NF],
                                start=True, stop=True)
                            pull(feed, quota)
                        at2 = p1.tile([P, 2 * NF], BF16, name="at2",
                                      tag="at", bufs=4)
                        nc.scalar.activation(at2[:, y0:], sc2[:, y0:],
                                             AF.Exp,
                                             scale=float(HD) ** -0.5)
                        if diag:
                            for hi in range(2):
                                nc.vector.tensor_mul(
                                    at2[:, hi * NF + y0:(hi + 1) * NF],
                                    at2[:, hi * NF + y0:(hi + 1) * NF],
                                    cmask[j // 2][:, (j % 2) * NF + y0:
                                                  (j % 2 + 1) * NF])
                        ats[(t, kt % 2)] = (at2, y0)
                    if kt >= 1:
                        for t in ts:
                            at2, py0 = ats[(t, (kt - 1) % 2)]
                            for hi, h in enumerate((t, t + NQT)):
                                nc.tensor.matmul(
                                    otps[h][:, py0:],
                                    v1[kt - 1][:, hi * (HD + 1):
                                               (hi + 1) * (HD + 1)],
                                    at2[:, hi * NF + py0:(hi + 1) * NF],
                                    start=(kt - 1 == 0), stop=False)
                for t in ts:
                    at2, py0 = ats[(t, (nk - 1) % 2)]
                    for hi, h in enumerate((t, t + NQT)):
                        nc.tensor.matmul(
                            otps[h][:, py0:],
                            v1[nk - 1][:, hi * (HD + 1):(hi + 1) * (HD + 1)],
                            at2[:, hi * NF + py0:(hi + 1) * NF],
                            start=(nk - 1 == 0), stop=True)
                # normalization: 1/denom -> broadcast matmul -> scale
                for t in ts:
                    for hi, h in enumerate((t, t + NQT)):
                        otp = otps[h]
                        rec = p1.tile([HD + 1, NF], F32, name="rec",
                                      tag="rec", bufs=2)
                        # full-tile op: the custom DVE op mis-handles
                        # partition-offset slices; rows 0-63 are junk
                        nc.vector.reciprocal_approx_fast(out=rec, in_=otp)
                        recb = p1.tile([P, NF], BF16, name="recb",
                                       tag="recb", bufs=2)
                        if recb_init[0] < 2:
                            # zero rows != 64 once per ring buffer so the
                            # broadcast matmul sees finite values
                            recb_init[0] += 1
                            nc.vector.memset(recb, 0.0)
                        nc.scalar.activation(recb[HD:HD + 1, :],
                                             rec[HD:HD + 1, :], AF.Copy)
                        # broadcast via a full 128x128-mode matmul (no PE
                        # mode switch): lhsT row 64 all-ones, rest zero
                        bc = psc.tile([P, NF], F32, name="bc", tag="sc")
                        nc.tensor.matmul(bc, ones_bc, recb,
                                         start=True, stop=True)
                        # DVE may read only ONE psum operand: stage in SBUF
                        bcs = p1.tile([HD, NF], F32, name="bcs", tag="bcs",
                                      bufs=2)
                        nc.vector.tensor_copy(bcs, bc[0:HD, :])
                        # deep ring: ao frees via its DRAM write, and the
                        # model DMA rings freeze during collectives
                        ao = p1.tile([HD, NF], BF16, name="ao", tag="ao",
                                     bufs=12)
                        nc.vector.tensor_mul(ao, otp[0:HD, :], bcs)
                        nc.sync.dma_start(
                            out=ag_in[t][(qi // 2) * P + hi * HD:
                                         (qi // 2) * P + (hi + 1) * HD,
                                         (qi % 2) * NF:(qi % 2 + 1) * NF],
                            in_=ao)

            def gather(t, h):
                nc.gpsimd.collective_compute(
                    "AllGather", mybir.AluOpType.bypass,
                    replica_groups=groups,
                    ins=[ag_in[t][h * P:(h + 1) * P, :].opt()],
                    outs=[ag_out[t][h * GROUP * P:(h + 1) * GROUP * P,
                                    :].opt()])

            # ---- phase schedule ----
            for _ in proj_chunk(0):
                pass
            for qi in range(N_CHUNKS - 1):
                if qi == 1:
                    drip_wo(0)       # 2MB wo quarters load during qi=1/2,
                if qi == 2:
                    drip_wo(1)       # well before collectives freeze DMA
                feed = proj_chunk(qi + 1)
                n_steps = 2 * GROUP * (qi + 1) * NF // P   # scores MMs
                n_feed = (NQT + 1) * D_TILES + (NF // P) * D_TILES
                quota = (n_feed + n_steps - 1) // n_steps
                for t in range(NQT):
                    att_block([t], qi, feed, quota)
                pull(feed, 10 ** 9)   # flush any remaining proj work
                if qi == 1:
                    for t in range(NQT):
                        gather(t, 0)   # first halves overlap qi=2 attention
            for tp in ((0, 1), (2, 3)):
                att_block(list(tp), N_CHUNKS - 1, None, 0)
                for t in tp:
                    gather(t, 1)

            pot_cm.__exit__(None, None, None)
            psc_cm.__exit__(None, None, None)
            pacc_cm.__exit__(None, None, None)
            p2_cm.__exit__(None, None, None)

            # ---- out projection ----
            with tc.tile_pool(name="oppsum", bufs=8, space="PSUM") as pop, \
                 tc.tile_pool(name="p5", bufs=1) as p5:
                for i in (2, 3):
                    wo_q[i] = p5.tile([P, (WO_R // P) * NF], BF16,
                                      name=f"wo_q{i}")
                    drip_wo(i)
                pid = nc.sync.partition_id()
                rk = pid % GROUP
                # rank r reads gathered half r//2, column block r%2
                row_base = (rk // 2) * (GROUP * P)
                colb = (rk % 2) * NF
                atf = {}
                for t in range(NQT):
                    for g in range(GROUP):
                        a = p5.tile([P, OUT_S], BF16, name=f"atf{t}_{g}")
                        nc.sync.dma_start(
                            out=a,
                            in_=ag_out[t][ds(row_base + g * P, P),
                                          ds(colb, OUT_S)])
                        atf[(t, g)] = a
                kts = [(t, g) for t in range(NQT) for g in range(GROUP)]
                # Split contraction: phase A = kidx 0-7 (pairs t0/t1, whose
                # gathers finish during the last attention superblock) for
                # ALL 16 output tiles, staged to bf16 SBUF partials. That
                # queues ~33us of ready matmul work ahead of the t2/t3
                # gather dependency, so the PE never idles (or goes HAM-
                # cold) waiting for the tail gathers. Phase B = kidx 8-15,
                # then partial + psum summed on DVE.
                partial = {}
                for n_pair in ((0, 1), (2, 3)):
                    ops = {(m, n_o): pop.tile([P, NF], F32, name="op",
                                              tag="op")
                           for m in range(OUT_S // P) for n_o in n_pair}
                    for kidx in range(8):
                        t, g = kts[kidx]
                        for m in range(OUT_S // P):
                            for n_o in n_pair:
                                nc.tensor.matmul(
                                    ops[(m, n_o)],
                                    atf[(t, g)][:, m * P:(m + 1) * P],
                                    wo_sb(kidx, n_o),
                                    start=(kidx == 0),
                                    stop=(kidx == 7))
                    for m in range(OUT_S // P):
                        for n_o in n_pair:
                            pt = p5.tile([P, NF], BF16, name="oppart",
                                         tag="oppart", bufs=16)
                            nc.vector.tensor_copy(pt, ops[(m, n_o)])
                            partial[(m, n_o)] = pt
                for n_pair in ((0, 1), (2, 3)):
                    ops = {(m, n_o): pop.tile([P, NF], F32, name="op2",
                                              tag="op")
                           for m in range(OUT_S // P) for n_o in n_pair}
                    for kidx in range(8, 16):
                        t, g = kts[kidx]
                        for m in range(OUT_S // P):
                            for n_o in n_pair:
                                nc.tensor.matmul(
                                    ops[(m, n_o)],
                                    atf[(t, g)][:, m * P:(m + 1) * P],
                                    wo_sb(kidx, n_o),
                                    start=(kidx == 8),
                                    stop=(kidx == 15))
                    for m in range(OUT_S // P):
                        for n_o in n_pair:
                            osb = p1.tile([P, NF], F32, name="osb",
                                          tag="osb", bufs=4)
                            nc.vector.tensor_add(osb, ops[(m, n_o)],
                                                 partial[(m, n_o)])
                            nc.sync.dma_start(
                                out=out_p[m * P:(m + 1) * P,
                                          n_o * NF:(n_o + 1) * NF],
                                in_=osb)
            p1_cm.__exit__(None, None, None)
            dram_pool.__exit__(None, None, None)
    nc.finalize()
    return nc


# ---------------------------------------------------------------------------
# host-side sharding / unsharding
# ---------------------------------------------------------------------------

def _local_head_perm(nhl):
    nqt = nhl // 2
    order = []
    for t in range(nqt):
        order.append(t)
        order.append(t + nqt)
    return order


def _tile_rows(w):
    """[D_TILES*P, C] -> [P, D_TILES*C]: block k = rows k*P..(k+1)*P."""
    d, c = w.shape
    k = d // P
    return np.ascontiguousarray(
        w.reshape(k, P, c).transpose(1, 0, 2).reshape(P, k * c))


def shard_inputs(x, Wq, Wk, Wv, Wo):
    import ml_dtypes
    dt_ = ml_dtypes.bfloat16
    perm = _local_head_perm(NHL)
    in_maps = []
    nqt = NHL // 2
    row_idx = []
    for t in range(nqt):
        for g in range(GROUP):
            for h in (g * NHL + t, g * NHL + t + nqt):
                row_idx.extend(range(h * HD, (h + 1) * HD))
    # wo: [P, 64*NF]; halves h: blocks i = (n_o - 2h)*16 + kidx
    wo_perm = Wo[row_idx, :].astype(np.float32)  # [2048, 2048]
    wo_blocks = np.zeros((P, (WO_R // P) * WO_C), dtype=dt_)
    i = 0
    for n_o in range(WO_C // NF):
        for kidx in range(WO_R // P):
            wo_blocks[:, i * NF:(i + 1) * NF] = wo_perm[
                kidx * P:(kidx + 1) * P, n_o * NF:(n_o + 1) * NF].astype(dt_)
            i += 1
    for c in range(N_CORES):
        b, rk = c // GROUP, c % GROUP
        col_idx = []
        for t in perm:
            h = rk * NHL + t
            col_idx.extend(range(h * HD, (h + 1) * HD))
        kv_cols = []
        for kvh in range(rk * NKVL, (rk + 1) * NKVL):
            kv_cols.extend(range(kvh * HD, (kvh + 1) * HD))
        # xT tiled: chunk n, block k = x[n*NF:(n+1)*NF, k*P:(k+1)*P].T
        xb = x[b]  # [S, DIM]
        xt = xb.reshape(N_CHUNKS, NF, D_TILES, P).transpose(3, 0, 2, 1)
        # xt[p, n, k, c] = x[n*NF+c, k*P+p] -> [P, N_CHUNKS*D_TILES*NF]
        xt = np.ascontiguousarray(xt.reshape(P, -1)).astype(dt_)
        in_maps.append({
            "xT": xt,
            "wq": _tile_rows(Wq[:, col_idx]).astype(dt_),
            "wk": _tile_rows(Wk[:, kv_cols]).astype(dt_),
            "wv": _tile_rows(Wv[:, kv_cols]).astype(dt_),
            "wo": wo_blocks,
        })
    return in_maps


def unshard_output(results):
    out = np.zeros((B, S, WO_C), dtype=np.float32)
    for c in range(N_CORES):
        b, rk = c // GROUP, c % GROUP
        out[b, rk * OUT_S:(rk + 1) * OUT_S, :] = results[c]["out"]
    return out


_NC_CACHE = {}


def kernel(x, mask=None, Wq=None, Wk=None, Wv=None, Wo=None):
    """Full-input entry point: returns [B, S, DIM] float32."""
    global LAST_RESULTS
    from concourse.bass_utils import run_bass_kernel_spmd

    x = np.asarray(x, dtype=np.float32)
    if "v2" not in _NC_CACHE:
        _NC_CACHE["v2"] = build_nc_v2()
    nc = _NC_CACHE["v2"]
    in_maps = shard_inputs(x, np.asarray(Wq), np.asarray(Wk),
                           np.asarray(Wv), np.asarray(Wo))
    res = run_bass_kernel_spmd(nc, in_maps, core_ids=list(range(N_CORES)),
                               trace=bool(os.environ.get("KERNEL_TRACE")))
    LAST_RESULTS = res
    return unshard_output(res.results)


# revision 18
# speedup vs baseline: 1.1698x; 1.0470x over previous
"""Distributed GQA attention kernel for 8 TRN2 NeuronCores — v2.

Problem: B=2, S=2048, DIM=2048, NH=32 q heads, NKV=8 kv heads, HD=64,
RoPE (base 10000), causal mask, out-projection.

Sharding (8 cores): core c -> batch b = c//4, rank r = c%4.
Each core: 8 q heads (as 4 pair-tiles), 2 kv heads, full S. Host
reassembles out[b, r*512:(r+1)*512, :] from core 4*b + r.

v2 structure (vs v1):
  - Host supplies x TRANSPOSED (xT [DIM, S] bf16): no on-chip transposes.
  - v projected directly in [key, dim] orientation (no vT transpose).
  - Scores matmuls run in 128x128 array mode with zero-padded kT copies
    (kTA/kTB) -> no PE tiling-mode switches, uniform warm matmul stream.
  - qi-outer attention interleaved with the NEXT s-chunk's projections:
    proj matmuls feed the PE queue between scores and PV so the PE never
    waits on the exp chain and HAM stays at full clock.
  - Softmax denominators: ones-column in v1 (PV computes sums), then
    reciprocal_approx_fast (DVE) + K=1 broadcast matmul (no DRAM hop).
  - Causal mask multiplies on DVE (gpsimd is reserved for collectives so
    they can't block compute).
  - Per-pair AllGather of normalized A^T overlapped with later pairs.
"""

import os
import numpy as np

import concourse.bass as bass
import concourse.mybir as mybir
from concourse import bacc, tile
from concourse.bass import ds

F32 = mybir.dt.float32
BF16 = mybir.dt.bfloat16
AF = mybir.ActivationFunctionType

# -------- problem constants (full size) --------
B, S, DIM = 2, 2048, 2048
NH, NKV, HD = 32, 8, 64
ROPE_BASE = 10000.0
N_CORES = 8
GROUP = 4                      # cores per batch group
NHL = NH // GROUP              # 8 local q heads
NKVL = NKV // GROUP            # 2 local kv heads
NQT = NHL // 2                 # 4 head-pair tiles
P = 128                        # partitions
NF = 512                       # free-dim tile (one PSUM bank of f32)
N_CHUNKS = S // NF             # 4 s-chunks
D_TILES = DIM // P             # 16 contraction tiles
KT_TOTAL = S // P              # 16 key tiles
OUT_S = S // GROUP             # 512 output rows per core
WO_R = GROUP * NHL * HD        # 2048
WO_C = DIM

LAST_RESULTS = None


def _rope_tables(S_):
    """cos table tiled to 128 partitions, and a sign-folded sin table:
    rows p with p%64 < 32 carry -sin (x1 half), else +sin (x2 half)."""
    inv_freq = 1.0 / (ROPE_BASE ** (np.arange(0, HD, 2, dtype=np.float64) / HD))
    t = np.arange(S_, dtype=np.float64)
    freqs = inv_freq[:, None] * t[None, :]          # [32, S]
    cos32 = np.cos(freqs).astype(np.float32)
    sin32 = np.sin(freqs).astype(np.float32)
    c128 = np.tile(cos32, (4, 1))                   # [128, S]
    s128 = np.tile(np.concatenate([-sin32, sin32], axis=0), (2, 1))
    return c128, s128


def build_nc_v2():
    import ml_dtypes
    nc = bacc.Bacc(None, target_bir_lowering=False, num_devices=N_CORES)

    # All inputs host-retiled so each SBUF destination loads with ONE big
    # dma_start (fixed cost ~1-2us per DMA makes many small loads brutal):
    #   xt[n][p, k*NF+c]   = x[n*NF+c, k*P+p]      (chunk n, d-tile k)
    #   wq[p, k*DQ+c]      = Wq[k*P+p, c]          etc for wk/wv
    #   wo[h][p, i*NF+c]   = Wo[kidx*P+p, n_o*NF+c], i = (n_o-2h)*16+kidx
    xT_p = nc.declare_dram_parameter("xT", [P, N_CHUNKS * D_TILES * NF], BF16,
                                     isOutput=False)
    wq_p = nc.declare_dram_parameter("wq", [P, D_TILES * NHL * HD], BF16,
                                     isOutput=False)
    wk_p = nc.declare_dram_parameter("wk", [P, D_TILES * NKVL * HD], BF16,
                                     isOutput=False)
    wv_p = nc.declare_dram_parameter("wv", [P, D_TILES * NKVL * HD], BF16,
                                     isOutput=False)
    wo_p = nc.declare_dram_parameter("wo", [P, (WO_R // P) * WO_C], BF16,
                                     isOutput=False)
    out_p = nc.declare_dram_parameter("out", [OUT_S, WO_C], F32, isOutput=True)

    cos_np, sin_np = _rope_tables(S)
    cos_d = nc.inline_tensor(cos_np, name="cos_tab")
    sin_d = nc.inline_tensor(sin_np, name="sin_tab")
    ones_bc_np = np.zeros((P, P), dtype=ml_dtypes.bfloat16)
    ones_bc_np[HD, :] = 1.0
    ones_bc_d = nc.inline_tensor(ones_bc_np, name="ones_bc")
    onesv_d = nc.inline_tensor(np.ones((P, 1), dtype=ml_dtypes.bfloat16),
                               name="ones_col")
    xx = np.arange(P)[:, None]
    yy = np.arange(NF)[None, :]
    mask_np = np.stack([(yy - xx - j * P >= 0) for j in range(NF // P)])
    mask2_np = np.stack([np.concatenate([mask_np[j], mask_np[j + 1]], axis=1)
                         for j in (0, 2)])
    mask_d = nc.inline_tensor(mask2_np.astype(ml_dtypes.bfloat16),
                              name="cmask")

    groups = [list(range(g * GROUP, (g + 1) * GROUP))
              for g in range(N_CORES // GROUP)]

    with tile.TileContext(nc) as tc:
        with tc.tile_pool(name="persist", bufs=1) as pp:
            # comm bounce buffers (DRAM)
            dram_pool = tc.tile_pool(name="dram", bufs=1, space="DRAM")
            dp = dram_pool.__enter__()
            # half-gather layout: ag_in[t] rows h*128.. = qi-half h's A^T
            # (cols (qi%2)*512..); ag_out[t] rows h*512.. = gathered half h
            ag_in = [dp.tile([2 * P, 2 * NF], BF16, name=f"ag_in{t}")
                     for t in range(NQT)]
            ag_out = [dp.tile([2 * GROUP * P, 2 * NF], BF16,
                              name=f"ag_out{t}") for t in range(NQT)]
            dum_in = dp.tile([1, 4], F32, name="dum_in")
            dum_out = dp.tile([GROUP, 4], F32, name="dum_out")

            # dummy gather first: absorbs inter-core launch skew while
            # nothing is pending (collectives freeze the model DMA rings,
            # so real gathers should never be the first sync point)
            nc.gpsimd.collective_compute(
                "AllGather", mybir.AluOpType.bypass, replica_groups=groups,
                ins=[dum_in.opt()], outs=[dum_out.opt()])

            ones_bc = pp.tile([P, P], BF16, name="ones_bc_sb")
            cmask = [pp.tile([P, 2 * NF], BF16, name=f"cmask{j}")
                     for j in range(2)]

            def load_consts():
                nc.sync.dma_start(out=ones_bc, in_=ones_bc_d[:, :])
                for j in range(2):
                    nc.sync.dma_start(out=cmask[j], in_=mask_d[j, :, :])

            qT = [pp.tile([P, S], BF16, name=f"qT{t}") for t in range(NQT)]
            # zero-padded per-head k tiles: kTp[0] has head-A dims on
            # partitions 0-63 (zeros elsewhere), kTp[1] head-B on 64-127
            kTp = [pp.tile([P, S], BF16, name=f"kTp{i}") for i in range(2)]
            nc.vector.memset(kTp[0][HD:P, :], 0.0)
            nc.vector.memset(kTp[1][0:HD, :], 0.0)
            # v1 tiles: [128 keys, 130]: cols 0-63 kv0, 64 ones, 65-128 kv1,
            # 129 ones (ones via memset: no tiny DMAs)
            v1 = [pp.tile([P, 2 * (HD + 1)], BF16, name=f"v1_{c}")
                  for c in range(KT_TOTAL)]
            for c in range(KT_TOTAL):
                nc.vector.memset(v1[c][:, HD:HD + 1], 1.0)
                nc.vector.memset(v1[c][:, 2 * HD + 1:2 * HD + 2], 1.0)

            p1_cm = tc.tile_pool(name="p1", bufs=1)
            p1 = p1_cm.__enter__()
            p2_cm = tc.tile_pool(name="p2", bufs=1)
            p2 = p2_cm.__enter__()
            pacc_cm = tc.tile_pool(name="accpsum", bufs=2, space="PSUM")
            pacc = pacc_cm.__enter__()
            psc_cm = tc.tile_pool(name="scpsum", bufs=2, space="PSUM")
            psc = psc_cm.__enter__()
            pot_cm = tc.tile_pool(name="otpsum", bufs=2, space="PSUM")
            pot = pot_cm.__enter__()

            # ---- input loads: one big dma_start per logical input ----
            # xt chunk tile: [128, 16*512]; slice k = d-tile k's s-chunk
            def load_x_chunk(n):
                xs = p2.tile([P, D_TILES * NF], BF16, name="xch", tag="xch",
                             bufs=2)
                nc.sync.dma_start(
                    out=xs, in_=xT_p[:, n * D_TILES * NF:(n + 1) * D_TILES * NF])
                return xs

            DQ = NHL * HD
            DKV = NKVL * HD
            wq_all = p2.tile([P, D_TILES * DQ], BF16, name="wq_all")
            wk_all = p2.tile([P, D_TILES * DKV], BF16, name="wk_all")
            wv_all = p2.tile([P, D_TILES * DKV], BF16, name="wv_all")
            xsl0 = p2.tile([P, D_TILES * NF], BF16, name="xch", tag="xch",
                           bufs=2)
            # x on the sync ring, weights on the scalar ring: the two HWDGE
            # rings run concurrently, halving time-to-first-matmul
            nc.sync.dma_start(out=xsl0, in_=xT_p[:, 0:D_TILES * NF])
            nc.scalar.dma_start(out=wq_all, in_=wq_p[:, :])
            nc.scalar.dma_start(out=wk_all, in_=wk_p[:, :])
            nc.scalar.dma_start(out=wv_all, in_=wv_p[:, :])
            load_consts()
            wq_sb = [wq_all[:, k * DQ:(k + 1) * DQ] for k in range(D_TILES)]
            wk_sb = [wk_all[:, k * DKV:(k + 1) * DKV] for k in range(D_TILES)]
            wv_sb = [wv_all[:, k * DKV:(k + 1) * DKV] for k in range(D_TILES)]
            xsl = {0: xsl0}

            c_all = p2.tile([P, S], F32, name="c_all")
            s_all = p2.tile([P, S], F32, name="s_all")
            nc.scalar.dma_start(out=c_all, in_=cos_d[:, :])
            nc.scalar.dma_start(out=s_all, in_=sin_d[:, :])

            def load_cs(n):
                return (c_all[:, n * NF:(n + 1) * NF],
                        s_all[:, n * NF:(n + 1) * NF])

            # Wo: four resident 2MB quarters (quarter = one n_o column
            # group). Quarters 0/1 load into p1 during attention; 2/3 load
            # into the out-projection pool after the proj-phase pool frees.
            wo_q = {}
            wo_q[0] = p1.tile([P, (WO_R // P) * NF], BF16, name="wo_q0")
            wo_q[1] = p1.tile([P, (WO_R // P) * NF], BF16, name="wo_q1")

            def wo_sb(kidx, n_o):
                return wo_q[n_o][:, kidx * NF:(kidx + 1) * NF]

            def drip_wo(i):
                nc.sync.dma_start(
                    out=wo_q[i],
                    in_=wo_p[:, i * (WO_R // P) * NF:
                             (i + 1) * (WO_R // P) * NF])

            # ---- projection chunk as a generator: yields after each PE op
            def proj_chunk(n):
                ncol = slice(n * NF, (n + 1) * NF)
                xs = xsl[n]
                if n + 1 < N_CHUNKS:
                    xsl[n + 1] = load_x_chunk(n + 1)  # prefetch
                c128, s128 = load_cs(n)
                # q pair tiles + k, m-outer k-inner (one psum bank each)
                for m in range(NQT + 1):
                    acc = pacc.tile([P, NF], F32, name="acc", tag="acc")
                    for k in range(D_TILES):
                        if m < NQT:
                            lhsT = wq_sb[k][:, m * P:(m + 1) * P]
                        else:
                            lhsT = wk_sb[k]
                        nc.tensor.matmul(acc, lhsT,
                                         xs[:, k * NF:(k + 1) * NF],
                                         start=(k == 0),
                                         stop=(k == D_TILES - 1))
                        yield
                    # rope: out = raw*cos + swap(raw)*sgn_sin
                    qraw = p2.tile([P, NF], F32, name="qraw", tag="qraw",
                                   bufs=2)
                    nc.scalar.activation(qraw, acc, AF.Copy)
                    qswp = p2.tile([P, NF], F32, name="qswp", tag="qswp",
                                   bufs=2)
                    h32 = HD // 2
                    for blk in range(4):
                        src = blk ^ 1
                        nc.sync.dma_start(
                            out=qswp[blk * h32:(blk + 1) * h32, :],
                            in_=qraw[src * h32:(src + 1) * h32, :])
                    tcos = p2.tile([P, NF], F32, name="tcos", tag="tcos",
                                   bufs=2)
                    tsin = p2.tile([P, NF], F32, name="tsin", tag="tsin",
                                   bufs=2)
                    nc.gpsimd.tensor_mul(tcos, qraw, c128)
                    nc.vector.tensor_mul(tsin, qswp, s128)
                    if m < NQT:
                        nc.vector.tensor_add(qT[m][:, ncol], tcos, tsin)
                    else:
                        # k: write rope'd halves into the zero-padded tiles
                        kc = p2.tile([P, NF], BF16, name="kc", tag="kc",
                                     bufs=2)
                        nc.vector.tensor_add(kc, tcos, tsin)
                        nc.vector.tensor_copy(kTp[0][0:HD, ncol],
                                              kc[0:HD, :])
                        nc.vector.tensor_copy(kTp[1][HD:P, ncol],
                                              kc[HD:P, :])
                # v directly in [key, dim] orientation
                for kc_i in range(NF // P):
                    vacc = pacc.tile([P, P], F32, name="vacc", tag="acc")
                    for k in range(D_TILES):
                        nc.tensor.matmul(
                            vacc,
                            xs[:, k * NF + kc_i * P:k * NF + (kc_i + 1) * P],
                            wv_sb[k], start=(k == 0),
                            stop=(k == D_TILES - 1))
                        yield
                    c = n * (NF // P) + kc_i
                    nc.scalar.activation(v1[c][:, 0:HD], vacc[:, 0:HD],
                                         AF.Copy)
                    nc.scalar.activation(v1[c][:, HD + 1:2 * HD + 1],
                                         vacc[:, HD:2 * HD], AF.Copy)

            def pull(gen, k):
                if gen is None:
                    return
                for _ in range(k):
                    try:
                        next(gen)
                    except StopIteration:
                        return

            # ---- attention for one (t, qi): 1-step-lag PV pipeline ----
            recb_init = [0]

            def att_block(ts, qi, feed, quota):
                """Attention for pairs `ts` at query block qi, kt-interleaved.
                Two pairs give the PE enough independent work to stay dense
                when there is no projection feed (qi=3)."""
                nk = (qi + 1) * NF // P
                otps = {}
                for i, t in enumerate(ts):
                    for h in (t, t + NQT):
                        pool, tag = (pot, "otp") if i == 0 else (pacc, "acc")
                        otps[h] = pool.tile([HD + 1, NF], F32, name="otp",
                                            tag=tag)
                ats = {}          # (t, kt % 2) -> {h: (at, y0)}
                for kt in range(nk):
                    diag = kt >= qi * NF // P
                    j = kt - qi * NF // P if diag else 0
                    y0 = j * P if diag else 0
                    for t in ts:
                        # both heads' scores into one 2-bank psum tile ->
                        # ONE batched EXP (halves the scalar fixed cost);
                        # the [NF, NF+y0) gap is exp'd but never read
                        sc2 = psc.tile([P, 2 * NF], F32, name="sc2",
                                       tag="sc")
                        for hi in range(2):
                            nc.tensor.matmul(
                                sc2[:, hi * NF + y0:(hi + 1) * NF],
                                kTp[hi][:, kt * P:(kt + 1) * P],
                                qT[t][:, qi * NF + y0:(qi + 1) * NF],
                                start=True, stop=True)
                            pull(feed, quota)
                        at2 = p1.tile([P, 2 * NF], BF16, name="at2",
                                      tag="at", bufs=6)
                        nc.scalar.activation(at2[:, y0:], sc2[:, y0:],
                                             AF.Exp,
                                             scale=float(HD) ** -0.5)
                        if diag:
                            for hi in range(2):
                                nc.vector.tensor_mul(
                                    at2[:, hi * NF + y0:(hi + 1) * NF],
                                    at2[:, hi * NF + y0:(hi + 1) * NF],
                                    cmask[j // 2][:, (j % 2) * NF + y0:
                                                  (j % 2 + 1) * NF])
                        ats[(t, kt % 2)] = (at2, y0)
                    if kt >= 1:
                        for t in ts:
                            at2, py0 = ats[(t, (kt - 1) % 2)]
                            for hi, h in enumerate((t, t + NQT)):
                                nc.tensor.matmul(
                                    otps[h][:, py0:],
                                    v1[kt - 1][:, hi * (HD + 1):
                                               (hi + 1) * (HD + 1)],
                                    at2[:, hi * NF + py0:(hi + 1) * NF],
                                    start=(kt - 1 == 0), stop=False)
                for t in ts:
                    at2, py0 = ats[(t, (nk - 1) % 2)]
                    for hi, h in enumerate((t, t + NQT)):
                        nc.tensor.matmul(
                            otps[h][:, py0:],
                            v1[nk - 1][:, hi * (HD + 1):(hi + 1) * (HD + 1)],
                            at2[:, hi * NF + py0:(hi + 1) * NF],
                            start=(nk - 1 == 0), stop=True)
                # normalization: 1/denom -> broadcast matmul -> scale
                for t in ts:
                    for hi, h in enumerate((t, t + NQT)):
                        otp = otps[h]
                        rec = p1.tile([HD + 1, NF], F32, name="rec",
                                      tag="rec", bufs=4)
                        # full-tile op: the custom DVE op mis-handles
                        # partition-offset slices; rows 0-63 are junk
                        nc.vector.reciprocal_approx_fast(out=rec, in_=otp)
                        recb = p1.tile([P, NF], BF16, name="recb",
                                       tag="recb", bufs=4)
                        if recb_init[0] < 4:
                            # zero rows != 64 once per ring buffer so the
                            # broadcast matmul sees finite values
                            recb_init[0] += 1
                            nc.vector.memset(recb, 0.0)
                        nc.scalar.activation(recb[HD:HD + 1, :],
                                             rec[HD:HD + 1, :], AF.Copy)
                        # broadcast via a full 128x128-mode matmul (no PE
                        # mode switch): lhsT row 64 all-ones, rest zero
                        bc = psc.tile([P, NF], F32, name="bc", tag="sc")
                        nc.tensor.matmul(bc, ones_bc, recb,
                                         start=True, stop=True)
                        # DVE may read only ONE psum operand: stage in SBUF
                        bcs = p1.tile([HD, NF], F32, name="bcs", tag="bcs",
                                      bufs=3)
                        nc.vector.tensor_copy(bcs, bc[0:HD, :])
                        # deep ring: ao frees via its DRAM write, and the
                        # model DMA rings freeze during collectives
                        ao = p1.tile([HD, NF], BF16, name="ao", tag="ao",
                                     bufs=12)
                        nc.vector.tensor_mul(ao, otp[0:HD, :], bcs)
                        nc.sync.dma_start(
                            out=ag_in[t][(qi // 2) * P + hi * HD:
                                         (qi // 2) * P + (hi + 1) * HD,
                                         (qi % 2) * NF:(qi % 2 + 1) * NF],
                            in_=ao)

            def gather(t, h):
                nc.gpsimd.collective_compute(
                    "AllGather", mybir.AluOpType.bypass,
                    replica_groups=groups,
                    ins=[ag_in[t][h * P:(h + 1) * P, :].opt()],
                    outs=[ag_out[t][h * GROUP * P:(h + 1) * GROUP * P,
                                    :].opt()])

            # ---- phase schedule ----
            for _ in proj_chunk(0):
                pass
            for qi in range(N_CHUNKS - 1):
                if qi == 1:
                    drip_wo(0)       # 2MB wo quarters load during qi=1/2,
                if qi == 2:
                    drip_wo(1)       # well before collectives freeze DMA
                feed = proj_chunk(qi + 1)
                n_steps = 2 * GROUP * (qi + 1) * NF // P   # scores MMs
                n_feed = (NQT + 1) * D_TILES + (NF // P) * D_TILES
                quota = (n_feed + n_steps - 1) // n_steps
                for t in range(NQT):
                    att_block([t], qi, feed, quota)
                pull(feed, 10 ** 9)   # flush any remaining proj work
                if qi == 1:
                    for t in range(NQT):
                        gather(t, 0)   # first halves overlap qi=2 attention
            for tp in ((0, 1), (2, 3)):
                att_block(list(tp), N_CHUNKS - 1, None, 0)
                for t in tp:
                    gather(t, 1)

            pot_cm.__exit__(None, None, None)
            psc_cm.__exit__(None, None, None)
            pacc_cm.__exit__(None, None, None)
            p2_cm.__exit__(None, None, None)

            # ---- out projection ----
            with tc.tile_pool(name="oppsum", bufs=8, space="PSUM") as pop, \
                 tc.tile_pool(name="p5", bufs=1) as p5:
                for i in (2, 3):
                    wo_q[i] = p5.tile([P, (WO_R // P) * NF], BF16,
                                      name=f"wo_q{i}")
                    drip_wo(i)
                pid = nc.sync.partition_id()
                rk = pid % GROUP
                # rank r reads gathered half r//2, column block r%2
                row_base = (rk // 2) * (GROUP * P)
                colb = (rk % 2) * NF
                atf = {}
                for t in range(NQT):
                    for g in range(GROUP):
                        a = p5.tile([P, OUT_S], BF16, name=f"atf{t}_{g}")
                        nc.sync.dma_start(
                            out=a,
                            in_=ag_out[t][ds(row_base + g * P, P),
                                          ds(colb, OUT_S)])
                        atf[(t, g)] = a
                kts = [(t, g) for t in range(NQT) for g in range(GROUP)]
                # Split contraction: phase A = kidx 0-7 (pairs t0/t1, whose
                # gathers finish during the last attention superblock) for
                # ALL 16 output tiles, staged to bf16 SBUF partials. That
                # queues ~33us of ready matmul work ahead of the t2/t3
                # gather dependency, so the PE never idles (or goes HAM-
                # cold) waiting for the tail gathers. Phase B = kidx 8-15,
                # then partial + psum summed on DVE.
                partial = {}
                for n_pair in ((0, 1), (2, 3)):
                    ops = {(m, n_o): pop.tile([P, NF], F32, name="op",
                                              tag="op")
                           for m in range(OUT_S // P) for n_o in n_pair}
                    for kidx in range(8):
                        t, g = kts[kidx]
                        for m in range(OUT_S // P):
                            for n_o in n_pair:
                                nc.tensor.matmul(
                                    ops[(m, n_o)],
                                    atf[(t, g)][:, m * P:(m + 1) * P],
                                    wo_sb(kidx, n_o),
                                    start=(kidx == 0),
                                    stop=(kidx == 7))
                    for m in range(OUT_S // P):
                        for n_o in n_pair:
                            pt = p5.tile([P, NF], BF16, name="oppart",
                                         tag="oppart", bufs=16)
                            nc.vector.tensor_copy(pt, ops[(m, n_o)])
                            partial[(m, n_o)] = pt
                for n_pair in ((0, 1), (2, 3)):
                    ops = {(m, n_o): pop.tile([P, NF], F32, name="op2",
                                              tag="op")
                           for m in range(OUT_S // P) for n_o in n_pair}
                    for kidx in range(8, 16):
                        t, g = kts[kidx]
                        for m in range(OUT_S // P):
                            for n_o in n_pair:
                                nc.tensor.matmul(
                                    ops[(m, n_o)],
                                    atf[(t, g)][:, m * P:(m + 1) * P],
                                    wo_sb(kidx, n_o),
                                    start=(kidx == 8),
                                    stop=(kidx == 15))
                    for m in range(OUT_S // P):
                        for n_o in n_pair:
                            osb = p1.tile([P, NF], F32, name="osb",
                                          tag="osb", bufs=4)
                            nc.vector.tensor_add(osb, ops[(m, n_o)],
                                                 partial[(m, n_o)])
                            nc.sync.dma_start(
                                out=out_p[m * P:(m + 1) * P,
                                          n_o * NF:(n_o + 1) * NF],
                                in_=osb)
            p1_cm.__exit__(None, None, None)
            dram_pool.__exit__(None, None, None)
    nc.finalize()
    return nc


# ---------------------------------------------------------------------------
# host-side sharding / unsharding
# ---------------------------------------------------------------------------

def _local_head_perm(nhl):
    nqt = nhl // 2
    order = []
    for t in range(nqt):
        order.append(t)
        order.append(t + nqt)
    return order


def _tile_rows(w):
    """[D_TILES*P, C] -> [P, D_TILES*C]: block k = rows k*P..(k+1)*P."""
    d, c = w.shape
    k = d // P
    return np.ascontiguousarray(
        w.reshape(k, P, c).transpose(1, 0, 2).reshape(P, k * c))


def shard_inputs(x, Wq, Wk, Wv, Wo):
    import ml_dtypes
    dt_ = ml_dtypes.bfloat16
    perm = _local_head_perm(NHL)
    in_maps = []
    nqt = NHL // 2
    row_idx = []
    for t in range(nqt):
        for g in range(GROUP):
            for h in (g * NHL + t, g * NHL + t + nqt):
                row_idx.extend(range(h * HD, (h + 1) * HD))
    # wo: [P, 64*NF]; halves h: blocks i = (n_o - 2h)*16 + kidx
    wo_perm = Wo[row_idx, :].astype(np.float32)  # [2048, 2048]
    wo_blocks = np.zeros((P, (WO_R // P) * WO_C), dtype=dt_)
    i = 0
    for n_o in range(WO_C // NF):
        for kidx in range(WO_R // P):
            wo_blocks[:, i * NF:(i + 1) * NF] = wo_perm[
                kidx * P:(kidx + 1) * P, n_o * NF:(n_o + 1) * NF].astype(dt_)
            i += 1
    for c in range(N_CORES):
        b, rk = c // GROUP, c % GROUP
        col_idx = []
        for t in perm:
            h = rk * NHL + t
            col_idx.extend(range(h * HD, (h + 1) * HD))
        kv_cols = []
        for kvh in range(rk * NKVL, (rk + 1) * NKVL):
            kv_cols.extend(range(kvh * HD, (kvh + 1) * HD))
        # xT tiled: chunk n, block k = x[n*NF:(n+1)*NF, k*P:(k+1)*P].T
        xb = x[b]  # [S, DIM]
        xt = xb.reshape(N_CHUNKS, NF, D_TILES, P).transpose(3, 0, 2, 1)
        # xt[p, n, k, c] = x[n*NF+c, k*P+p] -> [P, N_CHUNKS*D_TILES*NF]
        xt = np.ascontiguousarray(xt.reshape(P, -1)).astype(dt_)
        in_maps.append({
            "xT": xt,
            "wq": _tile_rows(Wq[:, col_idx]).astype(dt_),
            "wk": _tile_rows(Wk[:, kv_cols]).astype(dt_),
            "wv": _tile_rows(Wv[:, kv_cols]).astype(dt_),
            "wo": wo_blocks,
        })
    return in_maps


def unshard_output(results):
    out = np.zeros((B, S, WO_C), dtype=np.float32)
    for c in range(N_CORES):
        b, rk = c // GROUP, c % GROUP
        out[b, rk * OUT_S:(rk + 1) * OUT_S, :] = results[c]["out"]
    return out


_NC_CACHE = {}


def kernel(x, mask=None, Wq=None, Wk=None, Wv=None, Wo=None):
    """Full-input entry point: returns [B, S, DIM] float32."""
    global LAST_RESULTS
    from concourse.bass_utils import run_bass_kernel_spmd

    x = np.asarray(x, dtype=np.float32)
    if "v2" not in _NC_CACHE:
        _NC_CACHE["v2"] = build_nc_v2()
    nc = _NC_CACHE["v2"]
    in_maps = shard_inputs(x, np.asarray(Wq), np.asarray(Wk),
                           np.asarray(Wv), np.asarray(Wo))
    res = run_bass_kernel_spmd(nc, in_maps, core_ids=list(range(N_CORES)),
                               trace=bool(os.environ.get("KERNEL_TRACE")))
    LAST_RESULTS = res
    return unshard_output(res.results)
